# revision 1
# baseline (speedup 1.0000x reference)
"""GAT (2-layer, PyG-default) Trainium2 Bass kernel, 8-core SPMD.

Strategy:
  - Destinations (and their incoming edges) are partitioned across the 8
    cores: core k owns dst nodes [k*npc, (k+1)*npc).
  - Phase 0 (replicated on every core): h1 = x @ [W1 | W1@Asrc | W1@Adst]
    written to a DRAM node table T1[N, 576] = [h1(512) | al_src(8) |
    al_dst(8) | pad].  Replicating this matmul avoids a 100MB allgather.
  - L1 edge phase: edges are grouped by dst into chunks of 128 consecutive
    dst nodes; per chunk one bulk dma_gather pulls T1[src] for all its
    edges (2304B/edge); one-hot selection matrices (dst-local vs iota)
    route per-edge messages into a PSUM accumulator via PE matmuls:
        u[d,:] = sum_e sel[e,d] * p[e,h] * h1[src_e]      (unnormalized)
        z[d,h] = sum_e sel[e,h] * p[e,h]
    with p = exp(leakyrelu(al_src[src] + al_dst[dst])).  Softmax is done
    unnormalized (shift-invariance not needed in fp32 at these logit
    magnitudes) and normalized once per dst: out = u / z.
  - The chunk result is relu'd and immediately projected through
    W2_ext = [W2 | W2@a2src | W2@a2dst] into a second node table row
    tb2[128, 64], written to DRAM.  One AllGather shares the [6272,64]
    per-core tables; L2 edge phase repeats the same machinery with
    256B/edge gathers and a single head.
  - int16 gather indices can't span 50k rows, so each chunk's edges are
    split into a low-half / high-half group (by table row), each gathered
    with a different base AP.

Self-contained: only needs numpy + the concourse tree at /opt/trn_rl_repo.
"""

import hashlib
import math
import os
import sys

import numpy as np

for _p in ("/opt/trn_rl_repo",):
    if _p not in sys.path:
        sys.path.insert(0, _p)

import concourse.bacc as bacc
import concourse.bass as bass
import concourse.tile as tile
from concourse import mybir
from concourse.bass_utils import run_bass_kernel_spmd

F32 = mybir.dt.float32
BF16 = mybir.dt.bfloat16
I16 = mybir.dt.int16
AF = mybir.ActivationFunctionType
OP = mybir.AluOpType

N_CORES = 8


# ----------------------------------------------------------------------------
# Host-side edge planning
# ----------------------------------------------------------------------------

def _edge_plan(src_rows, dst, n_cores, npc, nch, split, nrows):
    """Group edges by (dst-core, dst-chunk-of-128, src-half) and lay out
    gather indices / dst-local arrays.

    src_rows: int64 [E] table row per edge.  dst: int64 [E] global dst.
    split: table rows >= split are gathered from a base-offset AP so the
    local index fits int16.

    Returns (k_lo[nch], k_hi[nch], toff[nch], TOT,
             idx16 [n_cores,128,8*TOT] int16, dl [n_cores,128,TOT] f32).
    Token t*128+p of a chunk lives at partition p, free col toff+t.
    Padding tokens gather row 0 and have dl = -1 (matches no dst).
    """
    core = dst // npc
    dloc = dst - core * npc
    chunk = dloc >> 7
    d128 = dloc & 127
    half = (src_rows >= split).astype(np.int64)
    lidx = src_rows - half * split
    assert lidx.min() >= 0 and lidx.max() < 32768
    assert lidx.max() < max(split, nrows - split)

    ngr = n_cores * nch * 2
    key = (core * nch + chunk) * 2 + half
    cnt = np.bincount(key, minlength=ngr).reshape(n_cores, nch, 2)
    kg = -(-cnt // 128)
    k_lo = kg[:, :, 0].max(axis=0)
    k_hi = kg[:, :, 1].max(axis=0)
    CNT = k_lo + k_hi
    toff = np.zeros(nch, np.int64)
    toff[1:] = np.cumsum(CNT)[:-1]
    TOT = int(CNT.sum())

    order = np.argsort(key, kind="stable")
    sk = key[order]
    gstart = np.zeros(ngr + 1, np.int64)
    np.cumsum(np.bincount(sk, minlength=ngr), out=gstart[1:])
    rank = np.arange(len(sk)) - gstart[sk]

    c_ = chunk[order]
    h_ = half[order]
    co_ = core[order]
    bs = toff[c_] + np.where(h_ == 1, k_lo[c_], 0)  # block start (128-token units)
    pos = bs * 128 + rank  # token position within the core's full layout

    dl = np.full((n_cores, 128, TOT), -1.0, np.float32)
    dl[co_, pos % 128, pos // 128] = d128[order].astype(np.float32)

    idx16 = np.zeros((n_cores, 16, 8 * TOT), np.int16)
    j = rank  # token index local to this gather block
    idx16[co_, j % 16, 8 * bs + j // 16] = lidx[order].astype(np.int16)
    idx16 = np.tile(idx16, (1, 8, 1))

    return (
        [int(v) for v in k_lo],
        [int(v) for v in k_hi],
        [int(v) for v in toff],
        TOT,
        idx16,
        dl,
    )


def _host_prep(x, edge_index, W1, att1_src, att1_dst, W2, att2_src, att2_dst):
    N, F = x.shape
    H, C = att1_src.shape
    HC = H * C
    NCLS = W2.shape[1]
    n_cores = N_CORES
    # 128-aligned dst partition: core k owns [k*npcp, (k+1)*npcp) ∩ [0, N)
    nch = -(-N // (n_cores * 128))
    npcp = nch * 128
    npc = npcp
    assert (n_cores - 1) * npcp < N <= n_cores * npcp

    src = np.concatenate([edge_index[0], np.arange(N, dtype=edge_index.dtype)])
    dst = np.concatenate([edge_index[1], np.arange(N, dtype=edge_index.dtype)])
    src = src.astype(np.int64)
    dst = dst.astype(np.int64)

    split1 = (N // 2 + 127) & ~127
    rows1 = n_cores * npcp  # >= N; pad rows zeroed on device
    plan1 = _edge_plan(src, dst, n_cores, npc, nch, split1, rows1)

    rows2 = n_cores * npcp
    split2 = (n_cores // 2) * npcp
    assert rows2 == rows1 and split2 == split1
    plan2 = plan1  # identity row map: L2 gather plan == L1 plan

    # Folded attention-logit weight columns: al_src = x @ (W1 @ blockdiag(a))
    Wa_s = np.einsum("fhc,hc->fh", W1.reshape(F, H, C), att1_src).astype(np.float32)
    Wa_d = np.einsum("fhc,hc->fh", W1.reshape(F, H, C), att1_dst).astype(np.float32)
    W1e = np.ascontiguousarray(
        np.concatenate([W1, Wa_s, Wa_d], axis=1), dtype=np.float32
    )  # [F, HC+2H]

    w2s = (W2 @ att2_src[0]).astype(np.float32)  # [HC]
    w2d = (W2 @ att2_dst[0]).astype(np.float32)
    W2e_flat = np.zeros((HC, 64), np.float32)
    W2e_flat[:, :NCLS] = W2
    W2e_flat[:, NCLS] = w2s
    W2e_flat[:, NCLS + 1] = w2d
    nslab = HC // 128
    W2e = np.ascontiguousarray(
        W2e_flat.reshape(nslab, 128, 64).transpose(1, 0, 2)
    )  # [128, nslab, 64]

    import ml_dtypes
    bf = ml_dtypes.bfloat16
    xT = np.ascontiguousarray(x.T).astype(bf)  # [F, N]
    W1e = W1e.astype(bf)
    iota = np.tile(np.arange(128, dtype=np.float32), (128, 1)).astype(bf)
    ident = np.eye(128, dtype=np.float32)

    cfg = dict(
        N=N, F=F, H=H, C=C, HC=HC, NCLS=NCLS, n_cores=n_cores, npc=npc,
        nch=nch, npcp=npcp, split1=split1, rows1=rows1, split2=split2,
        rows2=rows2, nslab=nslab,
        k1_lo=plan1[0], k1_hi=plan1[1], toff1=plan1[2], TOT1=plan1[3],
        k2_lo=plan2[0], k2_hi=plan2[1], toff2=plan2[2], TOT2=plan2[3],
    )
    shared = dict(xT=xT, W1e=W1e, W2e=W2e, iota=iota, ident=ident)
    per_core = [
        dict(g1idx=plan1[4][k], dl1=plan1[5][k].astype(bf))
        for k in range(n_cores)
    ]
    return cfg, shared, per_core


# ----------------------------------------------------------------------------
# Device program
# ----------------------------------------------------------------------------

def _build_program(cfg):
    N, F, H, HC, NCLS = cfg["N"], cfg["F"], cfg["H"], cfg["HC"], cfg["NCLS"]
    n_cores, nch, npcp = cfg["n_cores"], cfg["nch"], cfg["npcp"]
    rows1, split1 = cfg["rows1"], cfg["split1"]
    rows2, split2 = cfg["rows2"], cfg["split2"]
    nslab = cfg["nslab"]
    ROW1 = 640  # bf16 cols: [h1(512) | al_src f32-as-bf16-pairs(16) | pad]
    ROW2 = 128  # bf16 cols: [h2b(40) | al2 f32 pairs(4) | pad]
    assert F == 128 and HC % 128 == 0

    nc = bacc.Bacc("TRN2", target_bir_lowering=False, debug=False,
                   num_devices=n_cores)

    xT = nc.dram_tensor("xT", [F, N], BF16, kind="ExternalInput").ap()
    W1e = nc.dram_tensor("W1e", [F, HC + 2 * H], BF16, kind="ExternalInput").ap()
    W2e = nc.dram_tensor("W2e", [128, nslab, 64], F32, kind="ExternalInput").ap()
    iota_d = nc.dram_tensor("iota", [128, 128], BF16, kind="ExternalInput").ap()
    ident_d = nc.dram_tensor("ident", [128, 128], F32, kind="ExternalInput").ap()
    g1idx = nc.dram_tensor("g1idx", [128, 8 * cfg["TOT1"]], I16,
                           kind="ExternalInput").ap()
    dl1_d = nc.dram_tensor("dl1", [128, cfg["TOT1"]], BF16,
                           kind="ExternalInput").ap()

    T1 = nc.dram_tensor("T1", [rows1, ROW1], BF16).ap()
    tb2_own = nc.dram_tensor("tb2_own", [npcp, ROW2], BF16).ap()
    tb2_full = nc.dram_tensor("tb2_full", [rows2, ROW2], BF16,
                              addr_space="Shared").ap()
    out2 = nc.dram_tensor("out2", [npcp, NCLS], F32, kind="ExternalOutput").ap()

    tensors = dict(
        xT=xT, W1e=W1e, W2e=W2e, iota=iota_d, ident=ident_d,
        g1idx=g1idx, dl1=dl1_d,
        T1=T1, tb2_own=tb2_own, tb2_full=tb2_full, out2=out2,
    )
    repeat = cfg.get("repeat", 1)
    with tile.TileContext(nc) as tc:
        for _ in range(repeat):
            _emit(tc, cfg, tensors)
    nc.compile()
    return nc


def _emit(tc, cfg, t):
    nc = tc.nc
    N, F, H, HC, NCLS = cfg["N"], cfg["F"], cfg["H"], cfg["HC"], cfg["NCLS"]
    n_cores, nch, npc, npcp = cfg["n_cores"], cfg["nch"], cfg["npc"], cfg["npcp"]
    rows1, split1 = cfg["rows1"], cfg["split1"]
    rows2, split2 = cfg["rows2"], cfg["split2"]
    nslab = cfg["nslab"]
    ROW1, ROW2 = 640, 128
    NW1 = HC + 2 * H  # phase-0 matmul width

    NTB = n_cores * nch  # total 128-row tiles across the padded table
    with tc.tile_pool(name="consts", bufs=1) as cpool:
        W1e_sb = cpool.tile([128, NW1], BF16)
        nc.sync.dma_start(W1e_sb[:], t["W1e"][:, :])
        W2e_sb = cpool.tile([128, nslab, 64], F32)
        nc.sync.dma_start(W2e_sb[:], t["W2e"][:, :, :])
        iota_sb = cpool.tile([128, 128], BF16)
        nc.sync.dma_start(iota_sb[:], t["iota"][:, :])
        ident_sb = cpool.tile([128, 128], F32)
        nc.sync.dma_start(ident_sb[:], t["ident"][:, :])
        ident_bf = cpool.tile([128, 128], BF16)
        nc.vector.tensor_copy(ident_bf[:], ident_sb[:])
        ald1_all = cpool.tile([128, NTB, H], F32)  # al_dst for every node tile
        ald1_sb = cpool.tile([128, nch, H], F32)  # this core's slab
        ald2_sb = cpool.tile([128, nch, 1], F32)  # captured during L1 W2 stage
        nc.vector.memset(ald1_all[:], 0.0)

        # ---------------- Phase 0: node table T1 ----------------
        with (
            tc.tile_pool(name="p0", bufs=3) as pool,
            tc.tile_pool(name="p0ps", bufs=2, space="PSUM") as pps,
        ):
            zt = pool.tile([128, ROW1], BF16, tag="zero")
            nc.vector.memset(zt[:], 0.0)
            for r0 in range(N, rows1, 128):
                nc.sync.dma_start(t["T1"][r0 : min(r0 + 128, rows1), :],
                                  zt[: min(128, rows1 - r0), :])

            ntile = -(-N // 128)
            for i in range(ntile):
                m = min(128, N - i * 128)
                xt = pool.tile([128, 128], BF16, tag="xt")
                nc.sync.dma_start(xt[:, :m], t["xT"][:, i * 128 : i * 128 + m])
                ps = pps.tile([128, 1024], F32, tag="ps")  # 2 banks
                nc.tensor.matmul(ps[:m, 0:HC], lhsT=xt[:, :m],
                                 rhs=W1e_sb[:, 0:HC], start=True, stop=True)
                nc.tensor.matmul(ps[:m, 512 : 512 + 2 * H], lhsT=xt[:, :m],
                                 rhs=W1e_sb[:, HC : HC + 2 * H],
                                 start=True, stop=True)
                # bf16 row [h1(512) | al_src packed as f32 bitcast(2H cols)]
                row = pool.tile([128, HC + 2 * H], BF16, tag="row")
                nc.vector.tensor_copy(row[:m, 0:HC], ps[:m, 0:HC])
                nc.scalar.copy(row[:m, HC : HC + 2 * H].bitcast(F32),
                               ps[:m, 512 : 512 + H])
                # al_dst slab kept on-chip: tile i -> ald1_all[:, i, :]
                nc.scalar.copy(ald1_all[:m, i, :],
                               ps[:m, 512 + H : 512 + 2 * H])
                nc.sync.dma_start(
                    t["T1"][i * 128 : i * 128 + m, 0 : HC + 2 * H], row[:m, :]
                )

        # this core's al_dst slab: columns [pid*nch, pid*nch + nch)
        pid = nc.partition_id()
        nc.sync.dma_start(
            ald1_sb[:], ald1_all[:, bass.ds(pid * nch, nch), :]
        )

        if cfg.get("phases", "full") == "p0":
            return
        # ---------------- L1 edge phase ----------------
        _edge_phase(
            tc, cfg, layer=1,
            gather_src=t["T1"], grow=ROW1, gcols=HC,
            split=split1, rows=rows1,
            idx_d=t["g1idx"], dl_d=t["dl1"],
            k_lo=cfg["k1_lo"], k_hi=cfg["k1_hi"], toff=cfg["toff1"],
            ald_sb=ald1_sb, iota_sb=iota_sb, ident_sb=ident_sb,
            ident_bf=ident_bf, W2e_sb=W2e_sb, tb2_own=t["tb2_own"], out2=None,
            H=H, nslab=nslab, ald2_cap=ald2_sb, NCLS_=NCLS,
        )

        if cfg.get("phases", "full") == "p0+l1":
            return
        # ---------------- allgather ----------------
        if cfg.get("no_collective"):
            # timing-model builds only: stand-in DMA for the AllGather
            nc.sync.dma_start(t["tb2_full"][0:npcp, :], t["tb2_own"][:, :])
        else:
            nc.gpsimd.collective_compute(
                "AllGather",
                OP.bypass,
                replica_groups=[list(range(n_cores))],
                ins=[t["tb2_own"][:, :]],
                outs=[t["tb2_full"][:, :]],
            )

        if cfg.get("phases", "full") == "p0+l1+ag":
            return
        # ---------------- L2 edge phase ----------------
        _edge_phase(
            tc, cfg, layer=2,
            gather_src=t["tb2_full"], grow=ROW2, gcols=NCLS,
            split=split2, rows=rows2,
            idx_d=t["g1idx"], dl_d=t["dl1"],
            k_lo=cfg["k2_lo"], k_hi=cfg["k2_hi"], toff=cfg["toff2"],
            ald_sb=ald2_sb, iota_sb=iota_sb, ident_sb=ident_sb,
            ident_bf=ident_bf, W2e_sb=None, tb2_own=None, out2=t["out2"],
            H=1, nslab=nslab,
        )


def _edge_phase(tc, cfg, layer, gather_src, grow, gcols, split, rows,
                idx_d, dl_d, k_lo, k_hi, toff, ald_sb, iota_sb, ident_sb,
                ident_bf, W2e_sb, tb2_own, out2, H, nslab,
                ald2_cap=None, NCLS_=None):
    """One GAT message-passing layer over this core's dst chunks."""
    nc = tc.nc
    nch = cfg["nch"]
    HC, NCLS = cfg["HC"], cfg["NCLS"]
    C = HC // H if layer == 1 else NCLS
    # bf16 record: [feat(gcols) | al_src as f32 bitcast pairs(2H) | pad]
    als_off = HC if layer == 1 else NCLS
    agg_w = gcols if layer == 1 else NCLS + 1  # width of u-matmul rhs

    lo_ap = gather_src[0:split, :]
    hi_ap = gather_src[split:rows, :]

    with (
        tc.tile_pool(name=f"gt{layer}", bufs=2) as gpool,
        tc.tile_pool(name=f"meta{layer}", bufs=2) as mpool,
        tc.tile_pool(name=f"sel{layer}", bufs=2) as spool,
        tc.tile_pool(name=f"msg{layer}", bufs=3) as msgpool,
        tc.tile_pool(name=f"small{layer}", bufs=3) as smpool,
        tc.tile_pool(name=f"out{layer}", bufs=2) as opool,
        tc.tile_pool(name=f"ps_tr{layer}", bufs=2, space="PSUM") as pp_tr,
        tc.tile_pool(name=f"ps_ald{layer}", bufs=2, space="PSUM") as pp_ald,
        tc.tile_pool(name=f"ps_u{layer}", bufs=2, space="PSUM") as pp_u,
        tc.tile_pool(name=f"ps_z{layer}", bufs=1, space="PSUM") as pp_z,
    ):
        for c in range(nch):
            klo, khi = k_lo[c], k_hi[c]
            CNT = klo + khi
            assert 1 <= CNT <= 64 or layer == 2 and CNT <= 448
            off = toff[c]

            gt = gpool.tile([128, CNT, grow], BF16, tag="gt")
            idx = mpool.tile([128, 8 * CNT], I16, tag="idx")
            nc.sync.dma_start(idx[:], idx_d[:, 8 * off : 8 * (off + CNT)])
            dl = mpool.tile([128, CNT], BF16, tag="dl")
            nc.sync.dma_start(dl[:], dl_d[:, off : off + CNT])
            # Cap each dma_gather at GMAX subtiles (large single gathers
            # crash the runtime at ~1400+ indices).
            GMAX = 8
            for g0, g1, ap in ((0, klo, lo_ap), (klo, CNT, hi_ap)):
                for b0 in range(g0, g1, GMAX):
                    b1 = min(b0 + GMAX, g1)
                    nk = b1 - b0
                    nc.gpsimd.dma_gather(
                        gt[:, b0:b1, :], ap, idx[:, 8 * b0 : 8 * b1],
                        nk * 128, nk * 128, grow,
                    )

            sel = spool.tile([128, CNT, 128], BF16, tag="sel")
            nc.vector.tensor_tensor(
                sel[:],
                dl[:, :, None].to_broadcast([128, CNT, 128]),
                iota_sb[:, None, :].to_broadcast([128, CNT, 128]),
                op=OP.is_equal,
            )

            # per-edge al_dst via transposed selection matrices
            ps_ald = pp_ald.tile([128, CNT, H], F32, tag="ald")
            for tt in range(CNT):
                ps_tr = pp_tr.tile([128, 128], BF16, tag="tr")
                nc.tensor.transpose(ps_tr[:], sel[:, tt, :], ident_bf[:])
                seldm = smpool.tile([128, 128], F32, tag="seldm")
                nc.scalar.copy(seldm[:], ps_tr[:])
                nc.tensor.matmul(
                    ps_ald[:, tt, :], lhsT=seldm[:], rhs=ald_sb[:, c, :],
                    start=True, stop=True,
                )

            # p = exp(leakyrelu(al_src + al_dst))
            s_t = smpool.tile([128, CNT, H], F32, tag="s")
            nc.vector.tensor_tensor(
                s_t[:],
                gt[:, :, als_off : als_off + 2 * H].bitcast(F32),
                ps_ald[:], op=OP.add,
            )
            l_t = smpool.tile([128, CNT, H], F32, tag="l")
            nc.vector.scalar_tensor_tensor(
                l_t[:], s_t[:], 0.2, s_t[:], op0=OP.mult, op1=OP.max
            )
            p_t = smpool.tile([128, CNT, H], F32, tag="p")
            nc.scalar.activation(p_t[:], l_t[:], AF.Exp)
            p_bf = smpool.tile([128, CNT, H], BF16, tag="pbf")
            nc.vector.tensor_copy(p_bf[:], p_t[:])

            ps_u = pp_u.tile([128, 512], F32, tag="u")
            if layer == 1:
                ps_z = pp_z.tile([128, H], F32, tag="z")
            for tt in range(CNT):
                msg = msgpool.tile([128, agg_w], BF16, tag="msg")
                if layer == 1:
                    nc.vector.tensor_tensor(
                        msg[:].rearrange("p (h c) -> p h c", h=H),
                        gt[:, tt, 0:gcols].rearrange("p (h c) -> p h c", h=H),
                        p_bf[:, tt, :, None].to_broadcast([128, H, C]),
                        op=OP.mult,
                    )
                else:
                    nc.vector.tensor_tensor(
                        msg[:, 0:NCLS],
                        gt[:, tt, 0:NCLS],
                        p_bf[:, tt, :].to_broadcast([128, NCLS]),
                        op=OP.mult,
                    )
                    nc.vector.tensor_copy(msg[:, NCLS : NCLS + 1], p_bf[:, tt, :])
                nc.tensor.matmul(
                    ps_u[:, 0:agg_w], lhsT=sel[:, tt, :], rhs=msg[:, 0:agg_w],
                    start=(tt == 0), stop=(tt == CNT - 1),
                )
                if layer == 1:
                    nc.tensor.matmul(
                        ps_z[:], lhsT=sel[:, tt, :], rhs=p_bf[:, tt, :],
                        start=(tt == 0), stop=(tt == CNT - 1),
                    )

            if layer == 1:
                zb = smpool.tile([128, H], F32, tag="zb")
                nc.vector.tensor_scalar_max(zb[:], ps_z[:], 1e-30)
                rz = smpool.tile([128, H], F32, tag="rz")
                nc.vector.reciprocal(rz[:], zb[:])
                h2 = opool.tile([128, HC], F32, tag="h2")
                nc.vector.tensor_tensor(
                    h2[:].rearrange("p (h c) -> p h c", h=H),
                    ps_u[:, 0:HC].rearrange("p (h c) -> p h c", h=H),
                    rz[:, :, None].to_broadcast([128, H, C]),
                    op=OP.mult,
                )
                h2r = opool.tile([128, HC], F32, tag="h2r")
                nc.scalar.activation(h2r[:], h2[:], AF.Relu)
                # fused W2_ext projection -> tb2 row
                ps_o = pp_z.tile([128, 64], F32, tag="o")
                for j in range(nslab):
                    ps_tr = pp_tr.tile([128, 128], F32, tag="tr")
                    nc.tensor.transpose(
                        ps_tr[:], h2r[:, j * 128 : (j + 1) * 128], ident_sb[:]
                    )
                    h2t = smpool.tile([128, 128], F32, tag="h2t")
                    nc.scalar.copy(h2t[:], ps_tr[:])
                    nc.tensor.matmul(
                        ps_o[:], lhsT=h2t[:], rhs=W2e_sb[:, j, :],
                        start=(j == 0), stop=(j == nslab - 1),
                    )
                trow = opool.tile([128, 128], BF16, tag="trow")
                nc.vector.tensor_copy(trow[:, 0:NCLS_], ps_o[:, 0:NCLS_])
                nc.scalar.copy(
                    trow[:, NCLS_ : NCLS_ + 4].bitcast(F32),
                    ps_o[:, NCLS_ : NCLS_ + 2],
                )
                # capture this chunk's al2_dst column for the L2 phase
                nc.scalar.copy(ald2_cap[:, c, :],
                               ps_o[:, NCLS_ + 1 : NCLS_ + 2])
                nc.sync.dma_start(tb2_own[c * 128 : (c + 1) * 128, :], trow[:])
            else:
                zb = smpool.tile([128, 1], F32, tag="zb")
                nc.vector.tensor_scalar_max(zb[:], ps_u[:, NCLS : NCLS + 1], 1e-30)
                rz = smpool.tile([128, 1], F32, tag="rz")
                nc.vector.reciprocal(rz[:], zb[:])
                o2 = opool.tile([128, NCLS], F32, tag="o2")
                nc.vector.tensor_tensor(
                    o2[:], ps_u[:, 0:NCLS],
                    rz[:].to_broadcast([128, NCLS]), op=OP.mult,
                )
                nc.sync.dma_start(out2[c * 128 : (c + 1) * 128, :], o2[:])


# ----------------------------------------------------------------------------
# PJRT execution (with optional on-device iteration chaining for timing)
# ----------------------------------------------------------------------------

def _pjrt_exec(nc, in_maps, n_cores, iters=1, reps=3):
    """Like bass2jax.run_bass_via_pjrt but chains `iters` sequential
    executions of the NEFF inside one jit (iteration i+1 consumes iteration
    i's outputs as its donated output buffers), so (t[K]-t[1])/(K-1) measures
    pure on-device kernel time without host/transfer overhead."""
    import jax
    import numpy as _np
    from jax.sharding import Mesh, PartitionSpec
    from jax.experimental.shard_map import shard_map
    from concourse import bass2jax as b2j
    from concourse import mybir as _mb

    b2j.install_neuronx_cc_hook()
    partition_name = (nc.partition_id_tensor.name
                      if nc.partition_id_tensor else None)
    in_names, out_names, out_avals, zero_outs = [], [], [], []
    for alloc in nc.m.functions[0].allocations:
        if not isinstance(alloc, _mb.MemoryLocationSet):
            continue
        name = alloc.memorylocations[0].name
        if alloc.kind == "ExternalInput":
            if name != partition_name:
                in_names.append(name)
        elif alloc.kind == "ExternalOutput":
            shape = tuple(alloc.tensor_shape)
            dtype = _mb.dt.np(alloc.dtype)
            out_names.append(name)
            out_avals.append(jax.core.ShapedArray(shape, dtype))
            zero_outs.append(_np.zeros(shape, dtype))
    n_params = len(in_names)
    all_in_names = in_names + out_names
    if partition_name is not None:
        all_in_names = all_in_names + [partition_name]

    def _body(*args):
        ins = list(args[:n_params])
        zo = list(args[n_params:])
        for _ in range(iters):
            operands = ins + zo
            if partition_name is not None:
                operands.append(b2j.partition_id_tensor())
            outs = _bass_exec_bind(b2j, operands, out_avals, all_in_names,
                                   out_names, nc)
            zo = list(outs)
        return tuple(zo)

    devices = jax.devices()[:n_cores]
    mesh = Mesh(_np.asarray(devices), ("core",))
    in_specs = (PartitionSpec("core"),) * (n_params + len(out_names))
    out_specs = (PartitionSpec("core"),) * len(out_names)
    sharded = jax.jit(shard_map(_body, mesh=mesh, in_specs=in_specs,
                                out_specs=out_specs, check_rep=False),
                      keep_unused=True)
    concat_in = [
        _np.concatenate([_np.asarray(in_maps[c][nm]) for c in range(n_cores)],
                        axis=0)
        for nm in in_names
    ]
    concat_zeros = [_np.zeros((n_cores * z.shape[0], *z.shape[1:]), z.dtype)
                    for z in zero_outs]
    import time as _time
    from jax.sharding import NamedSharding
    sh = NamedSharding(mesh, PartitionSpec("core"))
    dev_in = [jax.device_put(a, sh) for a in concat_in]
    dev_zeros = [jax.device_put(a, sh) for a in concat_zeros]
    jax.block_until_ready(dev_in + dev_zeros)
    out_arrs = sharded(*dev_in, *dev_zeros)  # compile + run
    jax.block_until_ready(out_arrs)
    times = []
    for _ in range(reps):
        t0 = _time.perf_counter()
        out_arrs = sharded(*dev_in, *dev_zeros)
        jax.block_until_ready(out_arrs)
        times.append(_time.perf_counter() - t0)
    dt = min(times)
    results = [
        {nm: _np.asarray(out_arrs[i]).reshape(n_cores, *out_avals[i].shape)[c]
         for i, nm in enumerate(out_names)}
        for c in range(n_cores)
    ]
    return results, dt


def _bass_exec_bind(b2j, operands, out_avals, in_names, out_names, nc):
    return b2j._bass_exec_p.bind(
        *operands,
        out_avals=tuple(out_avals),
        in_names=tuple(in_names),
        out_names=tuple(out_names),
        lowering_input_output_aliases=(),
        sim_require_finite=True,
        sim_require_nnan=True,
        nc=nc,
    )


# ----------------------------------------------------------------------------
# Entry point
# ----------------------------------------------------------------------------

_CACHE = {}


def _run(inputs, trace=False):
    x = np.asarray(inputs["x"], np.float32)
    edge_index = np.asarray(inputs["edge_index"], np.int32)
    W1 = np.asarray(inputs["W1"], np.float32)
    a1s = np.asarray(inputs["att1_src"], np.float32)
    a1d = np.asarray(inputs["att1_dst"], np.float32)
    W2 = np.asarray(inputs["W2"], np.float32)
    a2s = np.asarray(inputs["att2_src"], np.float32)
    a2d = np.asarray(inputs["att2_dst"], np.float32)
    b1 = np.asarray(inputs["b1"], np.float32)
    b2 = np.asarray(inputs["b2"], np.float32)
    assert not b1.any() and not b2.any(), "nonzero bias unsupported"

    key = hashlib.sha1(
        b"v1" + edge_index.tobytes() + np.int64(x.shape).tobytes()
    ).hexdigest()
    cfg, shared, per_core = _host_prep(x, edge_index, W1, a1s, a1d, W2, a2s, a2d)
    if key not in _CACHE:
        _CACHE[key] = _build_program(cfg)
    nc = _CACHE[key]

    in_maps = []
    for k in range(cfg["n_cores"]):
        m = dict(shared)
        m.update(per_core[k])
        in_maps.append(m)
    res = run_bass_kernel_spmd(nc, in_maps, list(range(cfg["n_cores"])),
                               trace=trace)
    out = gather_out([res.results[k]["out2"] for k in range(cfg["n_cores"])],
                     cfg)
    return out.astype(np.float32), res


def gather_out(outs, cfg):
    N, npcp = cfg["N"], cfg["npcp"]
    return np.concatenate(
        [outs[k][: min(npcp, N - k * npcp)] for k in range(cfg["n_cores"])],
        axis=0,
    )


def kernel(**inputs):
    out, _ = _run(inputs, trace=False)
    return out



# revision 9
# speedup vs baseline: 1.1091x; 1.1091x over previous
"""GAT (2-layer, PyG-default) Trainium2 Bass kernel, 8-core SPMD.

Strategy:
  - Destinations (and their incoming edges) are partitioned across the 8
    cores: core k owns dst nodes [k*npc, (k+1)*npc).
  - Phase 0 (replicated on every core): h1 = x @ [W1 | W1@Asrc | W1@Adst]
    written to a DRAM node table T1[rows, 640] = [h1(512) | al_src f32
    pairs(16) | pad].  Replicating this matmul avoids a 100MB allgather.
    Additionally a small per-core dst table T_ald[npcp, 128] holds each
    OWN node's al_dst (f32 pairs in cols 0:16).
  - L1 edge phase: edges are grouped by dst into chunks of 128 consecutive
    dst nodes; per chunk one bulk dma_gather pulls T1[src] for all its
    edges (1280B/edge) and a second dma_gather pulls T_ald[dst] (256B/edge,
    chunk-local indices, no int16 split needed).  One-hot selection
    matrices sel[e,d] (built on DVE from pair-duplicated dst-local ids for
    the 2x 16-bit mode) route per-edge messages into PSUM via PE matmuls:
        u[d,:] = sum_e sel[e,d] * p[e,h] * h1[src_e]      (unnormalized)
        z[d,h] = sum_e sel[e,d] * p[e,h]
    with p = exp(leakyrelu(al_src[src] + al_dst[dst])).  Softmax is done
    unnormalized (shift-invariance not needed in fp32 at these logit
    magnitudes) and normalized once per dst: out = u / z.
  - The chunk result is relu'd and immediately projected through
    W2_ext = [W2 | W2@a2src | W2@a2dst] into a second node table row
    tb2[128, 128] (al2_src/al2_dst f32 pairs in cols 40:44), written to
    DRAM.  One AllGather shares the [npcp,128] per-core tables; L2 edge
    phase repeats the same machinery with 256B/edge gathers, a single
    head, and tb2_own as its dst table.
  - int16 gather indices can't span 50k rows, so each chunk's edges are
    split into a low-half / high-half group (by table row), each gathered
    with a different base AP.  The dst gathers are chunk-local (<32k rows)
    and need no split.

Self-contained: only needs numpy + the concourse tree at /opt/trn_rl_repo.
"""

import hashlib
import math
import os
import sys

import numpy as np

for _p in ("/opt/trn_rl_repo",):
    if _p not in sys.path:
        sys.path.insert(0, _p)

import concourse.bacc as bacc
import concourse.bass as bass
import concourse.tile as tile
from concourse import mybir
from concourse.bass_utils import run_bass_kernel_spmd

F32 = mybir.dt.float32
BF16 = mybir.dt.bfloat16
I16 = mybir.dt.int16
AF = mybir.ActivationFunctionType
OP = mybir.AluOpType

N_CORES = 8


# ----------------------------------------------------------------------------
# Host-side edge planning
# ----------------------------------------------------------------------------

def _edge_plan(src_rows, dst, n_cores, npc, nch, split, nrows):
    """Group edges by (dst-core, dst-chunk-of-128, src-half) and lay out
    gather indices / dst-local arrays.

    src_rows: int64 [E] table row per edge.  dst: int64 [E] global dst.
    split: table rows >= split are gathered from a base-offset AP so the
    local index fits int16.

    Returns (k_lo[nch], k_hi[nch], toff[nch], TOT,
             idx16 [n_cores,128,8*TOT] int16,    # src-gather indices
             gd16  [n_cores,128,8*TOT] int16,    # dst-gather indices (dloc)
             dl2   [n_cores,128,2*TOT] f32).     # pair-duplicated dst-local
    Token t*128+p of a chunk lives at partition p, free col toff+t.
    Padding tokens gather row 0 and have dl = -1 (matches no dst).
    """
    core = dst // npc
    dloc = dst - core * npc
    chunk = dloc >> 7
    d128 = dloc & 127
    half = (src_rows >= split).astype(np.int64)
    lidx = src_rows - half * split
    assert lidx.min() >= 0 and lidx.max() < 32768
    assert dloc.max() < 32768

    ngr = n_cores * nch * 2
    key = (core * nch + chunk) * 2 + half
    cnt = np.bincount(key, minlength=ngr).reshape(n_cores, nch, 2)
    kg = -(-cnt // 128)
    k_lo = kg[:, :, 0].max(axis=0)
    k_hi = kg[:, :, 1].max(axis=0)
    CNT = k_lo + k_hi
    toff = np.zeros(nch, np.int64)
    toff[1:] = np.cumsum(CNT)[:-1]
    TOT = int(CNT.sum())

    order = np.argsort(key, kind="stable")
    sk = key[order]
    gstart = np.zeros(ngr + 1, np.int64)
    np.cumsum(np.bincount(sk, minlength=ngr), out=gstart[1:])
    rank = np.arange(len(sk)) - gstart[sk]

    c_ = chunk[order]
    h_ = half[order]
    co_ = core[order]
    bs = toff[c_] + np.where(h_ == 1, k_lo[c_], 0)  # block start (128-token units)
    pos = bs * 128 + rank  # token position within the core's full layout

    dl = np.full((n_cores, 128, TOT), -1.0, np.float32)
    dl[co_, pos % 128, pos // 128] = d128[order].astype(np.float32)
    dl2 = np.repeat(dl, 2, axis=2)  # [n_cores, 128, 2*TOT] pair-duplicated

    j = rank  # token index local to this gather block
    idx16 = np.zeros((n_cores, 16, 8 * TOT), np.int16)
    idx16[co_, j % 16, 8 * bs + j // 16] = lidx[order].astype(np.int16)
    idx16 = np.tile(idx16, (1, 8, 1))
    gd16 = np.zeros((n_cores, 16, 8 * TOT), np.int16)
    gd16[co_, j % 16, 8 * bs + j // 16] = dloc[order].astype(np.int16)
    gd16 = np.tile(gd16, (1, 8, 1))

    return (
        [int(v) for v in k_lo],
        [int(v) for v in k_hi],
        [int(v) for v in toff],
        TOT,
        idx16,
        gd16,
        dl2,
    )


def _host_prep(x, edge_index, W1, att1_src, att1_dst, W2, att2_src, att2_dst):
    N, F = x.shape
    H, C = att1_src.shape
    HC = H * C
    NCLS = W2.shape[1]
    n_cores = N_CORES
    # 128-aligned dst partition: core k owns [k*npcp, (k+1)*npcp) ∩ [0, N)
    nch = -(-N // (n_cores * 128))
    npcp = nch * 128
    npc = npcp
    assert (n_cores - 1) * npcp < N <= n_cores * npcp

    src = np.concatenate([edge_index[0], np.arange(N, dtype=edge_index.dtype)])
    dst = np.concatenate([edge_index[1], np.arange(N, dtype=edge_index.dtype)])
    src = src.astype(np.int64)
    dst = dst.astype(np.int64)

    split1 = (N // 2 + 127) & ~127
    rows1 = n_cores * npcp  # >= N; pad rows zeroed on device
    plan1 = _edge_plan(src, dst, n_cores, npc, nch, split1, rows1)

    rows2 = n_cores * npcp
    split2 = (n_cores // 2) * npcp
    assert rows2 == rows1 and split2 == split1
    # identity row map: L2 gather plan == L1 plan

    # Folded attention-logit weight columns: al_src = x @ (W1 @ blockdiag(a))
    Wa_s = np.einsum("fhc,hc->fh", W1.reshape(F, H, C), att1_src).astype(np.float32)
    Wa_d = np.einsum("fhc,hc->fh", W1.reshape(F, H, C), att1_dst).astype(np.float32)
    W1e = np.ascontiguousarray(
        np.concatenate([W1, Wa_s, Wa_d], axis=1), dtype=np.float32
    )  # [F, HC+2H]

    w2s = (W2 @ att2_src[0]).astype(np.float32)  # [HC]
    w2d = (W2 @ att2_dst[0]).astype(np.float32)
    W2e_flat = np.zeros((HC, 64), np.float32)
    W2e_flat[:, :NCLS] = W2
    W2e_flat[:, NCLS] = w2s
    W2e_flat[:, NCLS + 1] = w2d
    nslab = HC // 128
    W2e = np.ascontiguousarray(
        W2e_flat.reshape(nslab, 128, 64).transpose(1, 0, 2)
    )  # [128, nslab, 64]

    import ml_dtypes
    bf = ml_dtypes.bfloat16
    xT = np.ascontiguousarray(x.T).astype(bf)  # [F, N]
    W1e = W1e.astype(bf)
    iota = np.tile(np.arange(128, dtype=np.float32), (128, 1)).astype(bf)
    ident = np.eye(128, dtype=np.float32)

    cfg = dict(
        N=N, F=F, H=H, C=C, HC=HC, NCLS=NCLS, n_cores=n_cores, npc=npc,
        nch=nch, npcp=npcp, split1=split1, rows1=rows1, split2=split2,
        rows2=rows2, nslab=nslab,
        k1_lo=plan1[0], k1_hi=plan1[1], toff1=plan1[2], TOT1=plan1[3],
        k2_lo=plan1[0], k2_hi=plan1[1], toff2=plan1[2], TOT2=plan1[3],
    )
    shared = dict(xT=xT, W1e=W1e, W2e=W2e.astype(bf), iota=iota, ident=ident)
    per_core = [
        dict(g1idx=plan1[4][k], gdidx=plan1[5][k], dl2=plan1[6][k].astype(bf))
        for k in range(n_cores)
    ]
    return cfg, shared, per_core


# ----------------------------------------------------------------------------
# Device program
# ----------------------------------------------------------------------------

def _build_program(cfg):
    N, F, H, HC, NCLS = cfg["N"], cfg["F"], cfg["H"], cfg["HC"], cfg["NCLS"]
    n_cores, nch, npcp = cfg["n_cores"], cfg["nch"], cfg["npcp"]
    rows1, split1 = cfg["rows1"], cfg["split1"]
    rows2, split2 = cfg["rows2"], cfg["split2"]
    nslab = cfg["nslab"]
    ROW1 = 640  # bf16 cols: [h1(512) | al_src f32-as-bf16-pairs(16) | pad]
    ROW2 = 128  # bf16 cols: [h2b(40) | al2 f32 pairs(4) | pad]
    assert F == 128 and HC % 128 == 0

    nc = bacc.Bacc("TRN2", target_bir_lowering=False, debug=False,
                   num_devices=n_cores)

    xT = nc.dram_tensor("xT", [F, N], BF16, kind="ExternalInput").ap()
    W1e = nc.dram_tensor("W1e", [F, HC + 2 * H], BF16, kind="ExternalInput").ap()
    W2e = nc.dram_tensor("W2e", [128, nslab, 64], BF16, kind="ExternalInput").ap()
    iota_d = nc.dram_tensor("iota", [128, 128], BF16, kind="ExternalInput").ap()
    ident_d = nc.dram_tensor("ident", [128, 128], F32, kind="ExternalInput").ap()
    g1idx = nc.dram_tensor("g1idx", [128, 8 * cfg["TOT1"]], I16,
                           kind="ExternalInput").ap()
    gdidx = nc.dram_tensor("gdidx", [128, 8 * cfg["TOT1"]], I16,
                           kind="ExternalInput").ap()
    dl2_d = nc.dram_tensor("dl2", [128, 2 * cfg["TOT1"]], BF16,
                           kind="ExternalInput").ap()

    T1 = nc.dram_tensor("T1", [rows1, ROW1], BF16).ap()
    T_ald = nc.dram_tensor("T_ald", [npcp, ROW2], BF16).ap()
    tb2_own = nc.dram_tensor("tb2_own", [npcp, ROW2], BF16).ap()
    tb2_full = nc.dram_tensor("tb2_full", [rows2, ROW2], BF16,
                              addr_space="Shared").ap()
    out2 = nc.dram_tensor("out2", [npcp, NCLS], F32, kind="ExternalOutput").ap()

    tensors = dict(
        xT=xT, W1e=W1e, W2e=W2e, iota=iota_d, ident=ident_d,
        g1idx=g1idx, gdidx=gdidx, dl2=dl2_d,
        T1=T1, T_ald=T_ald, tb2_own=tb2_own, tb2_full=tb2_full, out2=out2,
    )
    repeat = cfg.get("repeat", 1)
    with tile.TileContext(nc) as tc:
        for _ in range(repeat):
            _emit(tc, cfg, tensors)
    nc.compile()
    return nc


def _emit(tc, cfg, t):
    nc = tc.nc
    N, F, H, HC, NCLS = cfg["N"], cfg["F"], cfg["H"], cfg["HC"], cfg["NCLS"]
    n_cores, nch, npc, npcp = cfg["n_cores"], cfg["nch"], cfg["npc"], cfg["npcp"]
    rows1, split1 = cfg["rows1"], cfg["split1"]
    rows2, split2 = cfg["rows2"], cfg["split2"]
    nslab = cfg["nslab"]
    ROW1, ROW2 = 640, 128
    NW1 = HC + 2 * H  # phase-0 matmul width

    NTB = n_cores * nch  # total 128-row tiles across the padded table
    with tc.tile_pool(name="consts", bufs=1) as cpool:
        W1e_sb = cpool.tile([128, NW1], BF16)
        nc.sync.dma_start(W1e_sb[:], t["W1e"][:, :])
        W2e_sb = cpool.tile([128, nslab, 64], BF16)
        nc.sync.dma_start(W2e_sb[:], t["W2e"][:, :, :])
        iota_sb = cpool.tile([128, 128], BF16)
        nc.sync.dma_start(iota_sb[:], t["iota"][:, :])
        ident_sb = cpool.tile([128, 128], F32)
        nc.sync.dma_start(ident_sb[:], t["ident"][:, :])
        ident_bf = cpool.tile([128, 128], BF16)
        nc.vector.tensor_copy(ident_bf[:], ident_sb[:])
        ald1_all = cpool.tile([128, NTB, H], F32)  # al_dst for every node tile
        ald1_sb = cpool.tile([128, nch, H], F32)  # this core's slab
        nc.vector.memset(ald1_all[:], 0.0)

        # ---------------- Phase 0: node table T1 ----------------
        es0 = nc.enter_named_scope("p0", False)
        with (
            tc.tile_pool(name="p0", bufs=3) as pool,
            tc.tile_pool(name="p0ps", bufs=2, space="PSUM") as pps,
        ):
            zt = pool.tile([128, ROW1], BF16, tag="zero")
            nc.vector.memset(zt[:], 0.0)
            for r0 in range(N, rows1, 128):
                nc.sync.dma_start(t["T1"][r0 : min(r0 + 128, rows1), :],
                                  zt[: min(128, rows1 - r0), :])

            ntile = -(-N // 128)
            for i in range(ntile):
                m = min(128, N - i * 128)
                xt = pool.tile([128, 128], BF16, tag="xt")
                nc.sync.dma_start(xt[:, :m], t["xT"][:, i * 128 : i * 128 + m])
                ps = pps.tile([128, 1024], F32, tag="ps")  # 2 banks
                nc.tensor.matmul(ps[:m, 0:HC], lhsT=xt[:, :m],
                                 rhs=W1e_sb[:, 0:HC], start=True, stop=True)
                nc.tensor.matmul(ps[:m, 512 : 512 + 2 * H], lhsT=xt[:, :m],
                                 rhs=W1e_sb[:, HC : HC + 2 * H],
                                 start=True, stop=True)
                # bf16 row [h1(512) | al_src packed as f32 bitcast(2H cols)]
                row = pool.tile([128, HC + 2 * H], BF16, tag="row")
                nc.vector.tensor_copy(row[:m, 0 : HC // 2], ps[:m, 0 : HC // 2])
                nc.scalar.copy(row[:m, HC // 2 : HC], ps[:m, HC // 2 : HC])
                nc.vector.tensor_copy(row[:m, HC : HC + 2 * H].bitcast(F32),
                                      ps[:m, 512 : 512 + H])
                # al_dst slab kept on-chip: tile i -> ald1_all[:, i, :]
                nc.scalar.copy(ald1_all[:m, i, :],
                               ps[:m, 512 + H : 512 + 2 * H])
                nc.sync.dma_start(
                    t["T1"][i * 128 : i * 128 + m, 0 : HC + 2 * H], row[:m, :]
                )

        # this core's al_dst slab: columns [pid*nch, pid*nch + nch)
        pid = nc.partition_id()
        nc.sync.dma_start(
            ald1_sb[:], ald1_all[:, bass.ds(pid * nch, nch), :]
        )
        # dst table T_ald[npcp, 128]: own nodes' al_dst as f32 pairs
        with tc.tile_pool(name="tald", bufs=2) as apool:
            for c in range(nch):
                aldrow = apool.tile([128, 2 * H], BF16, tag="ar")
                nc.scalar.copy(aldrow[:].bitcast(F32), ald1_sb[:, c, :])
                nc.sync.dma_start(
                    t["T_ald"][c * 128 : (c + 1) * 128, 0 : 2 * H], aldrow[:]
                )
        nc.leave_named_scope("p0", es0[0], False)

        if cfg.get("phases", "full") == "p0":
            return
        # ---------------- L1 edge phase ----------------
        es1 = nc.enter_named_scope("l1", False)
        _edge_phase(
            tc, cfg, layer=1,
            gather_src=t["T1"], grow=ROW1, gcols=HC,
            split=split1, rows=rows1,
            idx_d=t["g1idx"], gdidx_d=t["gdidx"], dl2_d=t["dl2"],
            dald_src=t["T_ald"], ald_cols=(0, 2 * H),
            k_lo=cfg["k1_lo"], k_hi=cfg["k1_hi"], toff=cfg["toff1"],
            iota_sb=iota_sb, ident_bf=ident_bf,
            W2e_sb=W2e_sb, tb2_own=t["tb2_own"], out2=None,
            H=H, nslab=nslab, NCLS_=NCLS,
        )
        nc.leave_named_scope("l1", es1[0], False)

        if cfg.get("phases", "full") == "p0+l1":
            return
        # ---------------- allgather ----------------
        es_ag = nc.enter_named_scope("ag", False)
        if cfg.get("no_collective"):
            # timing-model builds only: stand-in DMA for the AllGather
            nc.sync.dma_start(t["tb2_full"][0:npcp, :], t["tb2_own"][:, :])
        else:
            nc.gpsimd.collective_compute(
                "AllGather",
                OP.bypass,
                replica_groups=[list(range(n_cores))],
                ins=[t["tb2_own"][:, :]],
                outs=[t["tb2_full"][:, :]],
            )
        nc.leave_named_scope("ag", es_ag[0], False)

        if cfg.get("phases", "full") == "p0+l1+ag":
            return
        # ---------------- L2 edge phase ----------------
        es2 = nc.enter_named_scope("l2", False)
        _edge_phase(
            tc, cfg, layer=2,
            gather_src=t["tb2_full"], grow=ROW2, gcols=NCLS,
            split=split2, rows=rows2,
            idx_d=t["g1idx"], gdidx_d=t["gdidx"], dl2_d=t["dl2"],
            dald_src=t["tb2_own"], ald_cols=(NCLS + 2, NCLS + 4),
            k_lo=cfg["k2_lo"], k_hi=cfg["k2_hi"], toff=cfg["toff2"],
            iota_sb=iota_sb, ident_bf=ident_bf,
            W2e_sb=None, tb2_own=None, out2=t["out2"],
            H=1, nslab=nslab,
        )
        nc.leave_named_scope("l2", es2[0], False)


def _edge_phase(tc, cfg, layer, gather_src, grow, gcols, split, rows,
                idx_d, gdidx_d, dl2_d, dald_src, ald_cols, k_lo, k_hi, toff,
                iota_sb, ident_bf, W2e_sb, tb2_own, out2, H, nslab,
                NCLS_=None):
    """One GAT message-passing layer over this core's dst chunks."""
    nc = tc.nc
    nch = cfg["nch"]
    HC, NCLS = cfg["HC"], cfg["NCLS"]
    C = HC // H if layer == 1 else NCLS
    # bf16 record: [feat(gcols) | al_src as f32 bitcast pairs(2H) | pad]
    als_off = HC if layer == 1 else NCLS
    agg_w = gcols if layer == 1 else NCLS + 1  # width of u-matmul rhs

    lo_ap = gather_src[0:split, :]
    hi_ap = gather_src[split:rows, :]
    GMAX = 8  # cap dma_gather at GMAX subtiles (runtime limit ~1400 idxs)

    with (
        tc.tile_pool(name=f"gt{layer}", bufs=2) as gpool,
        tc.tile_pool(name=f"ga{layer}", bufs=2) as gapool,
        tc.tile_pool(name=f"meta{layer}", bufs=2) as mpool,
        tc.tile_pool(name=f"sel{layer}", bufs=2) as spool,
        tc.tile_pool(name=f"msg{layer}", bufs=3) as msgpool,
        tc.tile_pool(name=f"small{layer}", bufs=3) as smpool,
        tc.tile_pool(name=f"out{layer}", bufs=2) as opool,
        tc.tile_pool(name=f"ps_tr{layer}", bufs=2, space="PSUM") as pp_tr,
        tc.tile_pool(name=f"ps_u{layer}", bufs=2, space="PSUM") as pp_u,
        tc.tile_pool(name=f"ps_z{layer}", bufs=1, space="PSUM") as pp_z,
    ):
        for c in range(nch):
            klo, khi = k_lo[c], k_hi[c]
            CNT = klo + khi
            assert 1 <= CNT <= 64
            off = toff[c]

            idx = mpool.tile([128, 8 * CNT], I16, tag="idx")
            nc.sync.dma_start(idx[:], idx_d[:, 8 * off : 8 * (off + CNT)])
            didx = mpool.tile([128, 8 * CNT], I16, tag="didx")
            nc.sync.dma_start(didx[:], gdidx_d[:, 8 * off : 8 * (off + CNT)])
            dl2 = mpool.tile([128, 2 * CNT], BF16, tag="dl2")
            nc.sync.dma_start(dl2[:], dl2_d[:, 2 * off : 2 * (off + CNT)])

            gt = gpool.tile([128, CNT, grow], BF16, tag="gt")
            for g0, g1, ap in ((0, klo, lo_ap), (klo, CNT, hi_ap)):
                for b0 in range(g0, g1, GMAX):
                    b1 = min(b0 + GMAX, g1)
                    nk = b1 - b0
                    nc.gpsimd.dma_gather(
                        gt[:, b0:b1, :], ap, idx[:, 8 * b0 : 8 * b1],
                        nk * 128, nk * 128, grow,
                    )
            ga = gapool.tile([128, CNT, 128], BF16, tag="ga")
            for b0 in range(0, CNT, GMAX):
                b1 = min(b0 + GMAX, CNT)
                nk = b1 - b0
                nc.gpsimd.dma_gather(
                    ga[:, b0:b1, :], dald_src[:, :], didx[:, 8 * b0 : 8 * b1],
                    nk * 128, nk * 128, 128,
                )

            # one-hot selection: sel[e, tt, d] = (dl[e,tt] == d), 2x DVE mode
            sel = spool.tile([128, CNT, 128], BF16, tag="sel")
            nc.vector.tensor_tensor(
                sel[:].rearrange("p t (q r) -> p t q r", r=2),
                dl2[:].rearrange("p (t r) -> p t r", r=2)[:, :, None, :]
                .to_broadcast([128, CNT, 64, 2]),
                iota_sb[:].rearrange("p (q r) -> p q r", r=2)[:, None, :, :]
                .to_broadcast([128, CNT, 64, 2]),
                op=OP.is_equal,
            )

            # p = exp(leakyrelu(al_src + al_dst)); al_dst via the dst gather
            s_t = smpool.tile([128, CNT, H], F32, tag="s")
            nc.vector.tensor_tensor(
                s_t[:],
                gt[:, :, als_off : als_off + 2 * H].bitcast(F32),
                ga[:, :, ald_cols[0] : ald_cols[1]].bitcast(F32),
                op=OP.add,
            )
            l_t = smpool.tile([128, CNT, H], F32, tag="l")
            nc.vector.scalar_tensor_tensor(
                l_t[:], s_t[:], 0.2, s_t[:], op0=OP.mult, op1=OP.max
            )
            pb = smpool.tile([128, CNT, H], BF16, tag="pb")
            nc.scalar.activation(pb[:], l_t[:], AF.Exp)
            p2 = smpool.tile([128, CNT, H, 2], BF16, tag="p2")
            nc.vector.tensor_copy(
                p2[:], pb[:, :, :, None].to_broadcast([128, CNT, H, 2])
            )

            ps_u = pp_u.tile([128, 512], F32, tag="u")
            if layer == 1:
                ps_z = pp_z.tile([128, H], F32, tag="z")
            for tt in range(CNT):
                msg = msgpool.tile([128, agg_w], BF16, tag="msg")
                if layer == 1:
                    nc.vector.tensor_tensor(
                        msg[:].rearrange("p (h q r) -> p h q r", h=H, r=2),
                        gt[:, tt, 0:gcols].rearrange("p (h q r) -> p h q r",
                                                     h=H, r=2),
                        p2[:, tt, :, None, :].to_broadcast([128, H, C // 2, 2]),
                        op=OP.mult,
                    )
                else:
                    nc.vector.tensor_tensor(
                        msg[:, 0:NCLS].rearrange("p (q r) -> p q r", r=2),
                        gt[:, tt, 0:NCLS].rearrange("p (q r) -> p q r", r=2),
                        p2[:, tt, 0, None, :].to_broadcast([128, NCLS // 2, 2]),
                        op=OP.mult,
                    )
                    nc.vector.tensor_copy(msg[:, NCLS : NCLS + 1], pb[:, tt, :])
                nc.tensor.matmul(
                    ps_u[:, 0:agg_w], lhsT=sel[:, tt, :], rhs=msg[:, 0:agg_w],
                    start=(tt == 0), stop=(tt == CNT - 1),
                )
                if layer == 1:
                    nc.tensor.matmul(
                        ps_z[:], lhsT=sel[:, tt, :], rhs=pb[:, tt, :],
                        start=(tt == 0), stop=(tt == CNT - 1),
                    )

            if layer == 1:
                zb = smpool.tile([128, H], F32, tag="zb")
                nc.vector.tensor_scalar_max(zb[:], ps_z[:], 1e-30)
                rz = smpool.tile([128, H], F32, tag="rz")
                nc.vector.reciprocal(rz[:], zb[:])
                h2 = opool.tile([128, HC], F32, tag="h2")
                nc.vector.tensor_tensor(
                    h2[:].rearrange("p (h c) -> p h c", h=H),
                    ps_u[:, 0:HC].rearrange("p (h c) -> p h c", h=H),
                    rz[:, :, None].to_broadcast([128, H, C]),
                    op=OP.mult,
                )
                h2r = opool.tile([128, HC], BF16, tag="h2r")
                nc.scalar.activation(h2r[:], h2[:], AF.Relu)
                # fused W2_ext projection -> tb2 row
                ps_o = pp_z.tile([128, 64], F32, tag="o")
                for j in range(nslab):
                    ps_tr = pp_tr.tile([128, 128], BF16, tag="tr")
                    nc.tensor.transpose(
                        ps_tr[:], h2r[:, j * 128 : (j + 1) * 128], ident_bf[:]
                    )
                    h2t = smpool.tile([128, 128], BF16, tag="h2t")
                    nc.scalar.copy(h2t[:], ps_tr[:])
                    nc.tensor.matmul(
                        ps_o[:], lhsT=h2t[:], rhs=W2e_sb[:, j, :],
                        start=(j == 0), stop=(j == nslab - 1),
                    )
                trow = opool.tile([128, 128], BF16, tag="trow")
                nc.vector.tensor_copy(trow[:, 0:NCLS_], ps_o[:, 0:NCLS_])
                nc.scalar.copy(
                    trow[:, NCLS_ : NCLS_ + 4].bitcast(F32),
                    ps_o[:, NCLS_ : NCLS_ + 2],
                )
                nc.sync.dma_start(tb2_own[c * 128 : (c + 1) * 128, :], trow[:])
            else:
                zb = smpool.tile([128, 1], F32, tag="zb")
                nc.vector.tensor_scalar_max(zb[:], ps_u[:, NCLS : NCLS + 1], 1e-30)
                rz = smpool.tile([128, 1], F32, tag="rz")
                nc.vector.reciprocal(rz[:], zb[:])
                o2 = opool.tile([128, NCLS], F32, tag="o2")
                nc.vector.tensor_tensor(
                    o2[:], ps_u[:, 0:NCLS],
                    rz[:].to_broadcast([128, NCLS]), op=OP.mult,
                )
                nc.sync.dma_start(out2[c * 128 : (c + 1) * 128, :], o2[:])


# ----------------------------------------------------------------------------
# PJRT execution (with optional on-device iteration chaining for timing)
# ----------------------------------------------------------------------------

def _pjrt_exec(nc, in_maps, n_cores, iters=1, reps=3):
    """Like bass2jax.run_bass_via_pjrt but chains `iters` sequential
    executions of the NEFF inside one jit (iteration i+1 consumes iteration
    i's outputs as its donated output buffers), so (t[K]-t[1])/(K-1) measures
    pure on-device kernel time without host/transfer overhead."""
    import jax
    import numpy as _np
    from jax.sharding import Mesh, PartitionSpec
    from jax.experimental.shard_map import shard_map
    from concourse import bass2jax as b2j
    from concourse import mybir as _mb

    b2j.install_neuronx_cc_hook()
    partition_name = (nc.partition_id_tensor.name
                      if nc.partition_id_tensor else None)
    in_names, out_names, out_avals, zero_outs = [], [], [], []
    for alloc in nc.m.functions[0].allocations:
        if not isinstance(alloc, _mb.MemoryLocationSet):
            continue
        name = alloc.memorylocations[0].name
        if alloc.kind == "ExternalInput":
            if name != partition_name:
                in_names.append(name)
        elif alloc.kind == "ExternalOutput":
            shape = tuple(alloc.tensor_shape)
            dtype = _mb.dt.np(alloc.dtype)
            out_names.append(name)
            out_avals.append(jax.core.ShapedArray(shape, dtype))
            zero_outs.append(_np.zeros(shape, dtype))
    n_params = len(in_names)
    all_in_names = in_names + out_names
    if partition_name is not None:
        all_in_names = all_in_names + [partition_name]

    def _body(*args):
        ins = list(args[:n_params])
        zo = list(args[n_params:])
        for _ in range(iters):
            operands = ins + zo
            if partition_name is not None:
                operands.append(b2j.partition_id_tensor())
            outs = _bass_exec_bind(b2j, operands, out_avals, all_in_names,
                                   out_names, nc)
            zo = list(outs)
        return tuple(zo)

    devices = jax.devices()[:n_cores]
    mesh = Mesh(_np.asarray(devices), ("core",))
    in_specs = (PartitionSpec("core"),) * (n_params + len(out_names))
    out_specs = (PartitionSpec("core"),) * len(out_names)
    sharded = jax.jit(shard_map(_body, mesh=mesh, in_specs=in_specs,
                                out_specs=out_specs, check_rep=False),
                      keep_unused=True)
    concat_in = [
        _np.concatenate([_np.asarray(in_maps[c][nm]) for c in range(n_cores)],
                        axis=0)
        for nm in in_names
    ]
    concat_zeros = [_np.zeros((n_cores * z.shape[0], *z.shape[1:]), z.dtype)
                    for z in zero_outs]
    import time as _time
    from jax.sharding import NamedSharding
    sh = NamedSharding(mesh, PartitionSpec("core"))
    dev_in = [jax.device_put(a, sh) for a in concat_in]
    dev_zeros = [jax.device_put(a, sh) for a in concat_zeros]
    jax.block_until_ready(dev_in + dev_zeros)
    out_arrs = sharded(*dev_in, *dev_zeros)  # compile + run
    jax.block_until_ready(out_arrs)
    times = []
    for _ in range(reps):
        t0 = _time.perf_counter()
        out_arrs = sharded(*dev_in, *dev_zeros)
        jax.block_until_ready(out_arrs)
        times.append(_time.perf_counter() - t0)
    dt = min(times)
    results = [
        {nm: _np.asarray(out_arrs[i]).reshape(n_cores, *out_avals[i].shape)[c]
         for i, nm in enumerate(out_names)}
        for c in range(n_cores)
    ]
    return results, dt


def _bass_exec_bind(b2j, operands, out_avals, in_names, out_names, nc):
    return b2j._bass_exec_p.bind(
        *operands,
        out_avals=tuple(out_avals),
        in_names=tuple(in_names),
        out_names=tuple(out_names),
        lowering_input_output_aliases=(),
        sim_require_finite=True,
        sim_require_nnan=True,
        nc=nc,
    )


# ----------------------------------------------------------------------------
# Entry point
# ----------------------------------------------------------------------------

_CACHE = {}


def _run(inputs, trace=False):
    x = np.asarray(inputs["x"], np.float32)
    edge_index = np.asarray(inputs["edge_index"], np.int32)
    W1 = np.asarray(inputs["W1"], np.float32)
    a1s = np.asarray(inputs["att1_src"], np.float32)
    a1d = np.asarray(inputs["att1_dst"], np.float32)
    W2 = np.asarray(inputs["W2"], np.float32)
    a2s = np.asarray(inputs["att2_src"], np.float32)
    a2d = np.asarray(inputs["att2_dst"], np.float32)
    b1 = np.asarray(inputs["b1"], np.float32)
    b2 = np.asarray(inputs["b2"], np.float32)
    assert not b1.any() and not b2.any(), "nonzero bias unsupported"

    key = hashlib.sha1(
        b"v2" + edge_index.tobytes() + np.int64(x.shape).tobytes()
    ).hexdigest()
    cfg, shared, per_core = _host_prep(x, edge_index, W1, a1s, a1d, W2, a2s, a2d)
    if key not in _CACHE:
        _CACHE[key] = _build_program(cfg)
    nc = _CACHE[key]

    in_maps = []
    for k in range(cfg["n_cores"]):
        m = dict(shared)
        m.update(per_core[k])
        in_maps.append(m)
    res = run_bass_kernel_spmd(nc, in_maps, list(range(cfg["n_cores"])),
                               trace=trace)
    out = gather_out([res.results[k]["out2"] for k in range(cfg["n_cores"])],
                     cfg)
    return out.astype(np.float32), res


def gather_out(outs, cfg):
    N, npcp = cfg["N"], cfg["npcp"]
    return np.concatenate(
        [outs[k][: min(npcp, N - k * npcp)] for k in range(cfg["n_cores"])],
        axis=0,
    )


def kernel(**inputs):
    out, _ = _run(inputs, trace=False)
    return out


# revision 19
# speedup vs baseline: 2.0244x; 1.8252x over previous
"""GAT (2-layer, PyG-default) Trainium2 Bass kernel, 8-core SPMD.

Strategy:
  - Destinations (and their incoming edges) are partitioned across the 8
    cores: core k owns dst nodes [k*npc, (k+1)*npc).
  - Phase 0 (replicated on every core): h1 = x @ [W1 | W1@Asrc | W1@Adst]
    written to a DRAM node table T1[rows, 640] = [h1(512) | al_src f32
    pairs(16) | pad].  Replicating this matmul avoids a 100MB allgather.
    Additionally a small per-core dst table T_ald[npcp, 128] holds each
    OWN node's al_dst (f32 pairs in cols 0:16).
  - L1 edge phase: edges are grouped by dst into chunks of 128 consecutive
    dst nodes; per chunk one bulk dma_gather pulls T1[src] for all its
    edges (1280B/edge) and a second dma_gather pulls T_ald[dst] (256B/edge,
    chunk-local indices, no int16 split needed).  One-hot selection
    matrices sel[e,d] (built on DVE from pair-duplicated dst-local ids for
    the 2x 16-bit mode) route per-edge messages into PSUM via PE matmuls:
        u[d,:] = sum_e sel[e,d] * p[e,h] * h1[src_e]      (unnormalized)
        z[d,h] = sum_e sel[e,d] * p[e,h]
    with p = exp(leakyrelu(al_src[src] + al_dst[dst])).  Softmax is done
    unnormalized (shift-invariance not needed in fp32 at these logit
    magnitudes) and normalized once per dst: out = u / z.
  - The chunk result is relu'd and immediately projected through
    W2_ext = [W2 | W2@a2src | W2@a2dst] into a second node table row
    tb2[128, 128] (al2_src/al2_dst f32 pairs in cols 40:44), written to
    DRAM.  One AllGather shares the [npcp,128] per-core tables; L2 edge
    phase repeats the same machinery with 256B/edge gathers, a single
    head, and tb2_own as its dst table.
  - int16 gather indices can't span 50k rows, so each chunk's edges are
    split into a low-half / high-half group (by table row), each gathered
    with a different base AP.  The dst gathers are chunk-local (<32k rows)
    and need no split.

Self-contained: only needs numpy + the concourse tree at /opt/trn_rl_repo.
"""

import hashlib
import math
import os
import sys

import numpy as np

for _p in ("/opt/trn_rl_repo",):
    if _p not in sys.path:
        sys.path.insert(0, _p)

import concourse.bacc as bacc
import concourse.bass as bass
import concourse.tile as tile
from concourse import mybir
from concourse.bass_utils import run_bass_kernel_spmd

F32 = mybir.dt.float32
BF16 = mybir.dt.bfloat16
FP8 = mybir.dt.float8e4
I16 = mybir.dt.int16
AF = mybir.ActivationFunctionType
OP = mybir.AluOpType

N_CORES = 8
_PAD_NEG = False  # pad gather slots use idx -1 (skip fetch) when True


# ----------------------------------------------------------------------------
# Host-side edge planning
# ----------------------------------------------------------------------------

def _edge_plan(src_rows, dst, n_cores, npc, nch, split, nrows):
    """Group edges by (dst-core, dst-chunk-of-128, src-half) and lay out
    gather indices / dst-local arrays.

    src_rows: int64 [E] table row per edge.  dst: int64 [E] global dst.
    split: table rows >= split are gathered from a base-offset AP so the
    local index fits int16.

    Returns (k_lo[nch], k_hi[nch], toff[nch], TOT,
             idx16 [n_cores,128,8*TOT] int16,    # src-gather indices
             gd16  [n_cores,128,8*TOT] int16,    # dst-gather indices (dloc)
             dl2   [n_cores,128,2*TOT] f32).     # pair-duplicated dst-local
    Token t*128+p of a chunk lives at partition p, free col toff+t.
    Padding tokens gather row 0 and have dl = -1 (matches no dst).
    """
    core = dst // npc
    dloc = dst - core * npc
    chunk = dloc >> 7
    d128 = dloc & 127
    half = (src_rows >= split).astype(np.int64)
    lidx = src_rows - half * split
    assert lidx.min() >= 0 and lidx.max() < 32768
    assert dloc.max() < 32768

    ngr = n_cores * nch * 2
    key = (core * nch + chunk) * 2 + half
    cnt = np.bincount(key, minlength=ngr).reshape(n_cores, nch, 2)
    kg = -(-cnt // 128)
    k_lo = kg[:, :, 0].max(axis=0)
    k_hi = kg[:, :, 1].max(axis=0)
    CNT = k_lo + k_hi
    toff = np.zeros(nch, np.int64)
    toff[1:] = np.cumsum(CNT)[:-1]
    TOT = int(CNT.sum())

    order = np.argsort(key, kind="stable")
    sk = key[order]
    gstart = np.zeros(ngr + 1, np.int64)
    np.cumsum(np.bincount(sk, minlength=ngr), out=gstart[1:])
    rank = np.arange(len(sk)) - gstart[sk]

    c_ = chunk[order]
    h_ = half[order]
    co_ = core[order]
    bs = toff[c_] + np.where(h_ == 1, k_lo[c_], 0)  # block start (128-token units)
    pos = bs * 128 + rank  # token position within the core's full layout

    dl = np.full((n_cores, 128, TOT), -1.0, np.float32)
    dl[co_, pos % 128, pos // 128] = d128[order].astype(np.float32)
    dl2 = np.repeat(dl, 2, axis=2)  # [n_cores, 128, 2*TOT] pair-duplicated

    j = rank  # token index local to this gather block
    # Pad slots get idx -1: trailing negatives are skipped by the gather
    # (no fetch, SBUF slot keeps old finite data).  Chunks 0/1 read
    # never-written pool bufs, so their pads fetch row 0 instead.
    idx16 = np.full((n_cores, 16, 8 * TOT), -1, np.int16)
    idx16[co_, j % 16, 8 * bs + j // 16] = lidx[order].astype(np.int16)
    head = 8 * (toff[2] if nch > 2 and _PAD_NEG else TOT)
    r = idx16[:, :, :head]
    r[r == -1] = 0
    idx16 = np.tile(idx16, (1, 8, 1))
    gd16 = np.zeros((n_cores, 16, 8 * TOT), np.int16)
    gd16[co_, j % 16, 8 * bs + j // 16] = dloc[order].astype(np.int16)
    gd16 = np.tile(gd16, (1, 8, 1))

    return (
        [int(v) for v in k_lo],
        [int(v) for v in k_hi],
        [int(v) for v in toff],
        TOT,
        idx16,
        gd16,
        dl2,
    )


def _host_prep(x, edge_index, W1, att1_src, att1_dst, W2, att2_src, att2_dst):
    N, F = x.shape
    H, C = att1_src.shape
    HC = H * C
    NCLS = W2.shape[1]
    n_cores = N_CORES
    # 128-aligned dst partition: core k owns [k*npcp, (k+1)*npcp) ∩ [0, N)
    nch = -(-N // (n_cores * 128))
    npcp = nch * 128
    npc = npcp
    assert (n_cores - 1) * npcp < N <= n_cores * npcp

    src = np.concatenate([edge_index[0], np.arange(N, dtype=edge_index.dtype)])
    dst = np.concatenate([edge_index[1], np.arange(N, dtype=edge_index.dtype)])
    src = src.astype(np.int64)
    dst = dst.astype(np.int64)

    split1 = (N // 2 + 127) & ~127
    rows1 = n_cores * npcp  # >= N; pad rows zeroed on device
    plan1 = _edge_plan(src, dst, n_cores, npc, nch, split1, rows1)

    rows2 = n_cores * npcp
    split2 = (n_cores // 2) * npcp
    assert rows2 == rows1 and split2 == split1
    # identity row map: L2 gather plan == L1 plan

    # Folded attention-logit weight columns: al_src = x @ (W1 @ blockdiag(a))
    Wa_s = np.einsum("fhc,hc->fh", W1.reshape(F, H, C), att1_src).astype(np.float32)
    Wa_d = np.einsum("fhc,hc->fh", W1.reshape(F, H, C), att1_dst).astype(np.float32)
    W1e = np.ascontiguousarray(
        np.concatenate([W1, Wa_s, Wa_d], axis=1), dtype=np.float32
    )  # [F, HC+2H]

    w2s = (W2 @ att2_src[0]).astype(np.float32)  # [HC]
    w2d = (W2 @ att2_dst[0]).astype(np.float32)
    W2e_flat = np.zeros((HC, 64), np.float32)
    W2e_flat[:, :NCLS] = W2
    W2e_flat[:, NCLS] = w2s
    W2e_flat[:, NCLS + 1] = w2d
    nslab = HC // 128
    W2e = np.ascontiguousarray(
        W2e_flat.reshape(nslab, 128, 64).transpose(1, 0, 2)
    )  # [128, nslab, 64]

    import ml_dtypes
    bf = ml_dtypes.bfloat16
    xT = np.ascontiguousarray(x.T).astype(bf)  # [F, N]
    W1e = W1e.astype(bf)
    iota = np.tile(np.arange(128, dtype=np.float32), (128, 1)).astype(bf)
    ident = np.eye(128, dtype=np.float32)

    cfg = dict(
        nq=4,
        N=N, F=F, H=H, C=C, HC=HC, NCLS=NCLS, n_cores=n_cores, npc=npc,
        nch=nch, npcp=npcp, split1=split1, rows1=rows1, split2=split2,
        rows2=rows2, nslab=nslab,
        k1_lo=plan1[0], k1_hi=plan1[1], toff1=plan1[2], TOT1=plan1[3],
        k2_lo=plan1[0], k2_hi=plan1[1], toff2=plan1[2], TOT2=plan1[3],
    )
    shared = dict(xT=xT, W1e=W1e, W2e=W2e.astype(bf), iota=iota, ident=ident)
    per_core = [
        dict(g1idx=plan1[4][k], gdidx=plan1[5][k], dl2=plan1[6][k].astype(bf))
        for k in range(n_cores)
    ]
    return cfg, shared, per_core


# ----------------------------------------------------------------------------
# Device program
# ----------------------------------------------------------------------------

def _build_program(cfg):
    N, F, H, HC, NCLS = cfg["N"], cfg["F"], cfg["H"], cfg["HC"], cfg["NCLS"]
    n_cores, nch, npcp = cfg["n_cores"], cfg["nch"], cfg["npcp"]
    rows1, split1 = cfg["rows1"], cfg["split1"]
    rows2, split2 = cfg["rows2"], cfg["split2"]
    nslab = cfg["nslab"]
    fp8 = cfg.get("t1_fp8", False)
    ROW1 = 768 if fp8 else 640  # T1 row: [h1(512) | al_src f32 bitcast | pad]
    ROW2 = 128  # bf16 cols: [h2b(40) | al2 f32 pairs(4) | pad]
    assert F == 128 and HC % 128 == 0

    nc = bacc.Bacc("TRN2", target_bir_lowering=False, debug=False,
                   num_devices=n_cores,
                   num_swdge_queues=cfg.get("nq", 1))

    xT = nc.dram_tensor("xT", [F, N], BF16, kind="ExternalInput").ap()
    W1e = nc.dram_tensor("W1e", [F, HC + 2 * H], BF16, kind="ExternalInput").ap()
    W2e = nc.dram_tensor("W2e", [128, nslab, 64], BF16, kind="ExternalInput").ap()
    iota_d = nc.dram_tensor("iota", [128, 128], BF16, kind="ExternalInput").ap()
    ident_d = nc.dram_tensor("ident", [128, 128], F32, kind="ExternalInput").ap()
    g1idx = nc.dram_tensor("g1idx", [128, 8 * cfg["TOT1"]], I16,
                           kind="ExternalInput").ap()
    gdidx = nc.dram_tensor("gdidx", [128, 8 * cfg["TOT1"]], I16,
                           kind="ExternalInput").ap()
    dl2_d = nc.dram_tensor("dl2", [128, 2 * cfg["TOT1"]], BF16,
                           kind="ExternalInput").ap()

    T1 = nc.dram_tensor("T1", [rows1, ROW1], FP8 if fp8 else BF16).ap()
    T_ald = nc.dram_tensor("T_ald", [npcp, ROW2], BF16).ap()
    tb2_own = nc.dram_tensor("tb2_own", [npcp, ROW2], BF16).ap()
    tb2_full = nc.dram_tensor("tb2_full", [rows2, ROW2], BF16,
                              addr_space="Shared").ap()
    out2 = nc.dram_tensor("out2", [npcp, NCLS], F32, kind="ExternalOutput").ap()

    tensors = dict(
        xT=xT, W1e=W1e, W2e=W2e, iota=iota_d, ident=ident_d,
        g1idx=g1idx, gdidx=gdidx, dl2=dl2_d,
        T1=T1, T_ald=T_ald, tb2_own=tb2_own, tb2_full=tb2_full, out2=out2,
    )
    repeat = cfg.get("repeat", 1)
    with tile.TileContext(nc) as tc:
        for _ in range(repeat):
            _emit(tc, cfg, tensors)
    nc.compile()
    return nc


def _emit(tc, cfg, t):
    nc = tc.nc
    N, F, H, HC, NCLS = cfg["N"], cfg["F"], cfg["H"], cfg["HC"], cfg["NCLS"]
    n_cores, nch, npc, npcp = cfg["n_cores"], cfg["nch"], cfg["npc"], cfg["npcp"]
    rows1, split1 = cfg["rows1"], cfg["split1"]
    rows2, split2 = cfg["rows2"], cfg["split2"]
    nslab = cfg["nslab"]
    fp8 = cfg.get("t1_fp8", False)
    ROW1, ROW2 = (768 if fp8 else 640), 128
    T1D = FP8 if fp8 else BF16
    nals = 4 * H if fp8 else 2 * H  # al_src f32 span in T1-dtype units
    NW1 = HC + 2 * H  # phase-0 matmul width

    NTB = n_cores * nch  # total 128-row tiles across the padded table
    with tc.tile_pool(name="consts", bufs=1) as cpool:
        W1e_sb = cpool.tile([128, NW1], BF16)
        nc.sync.dma_start(W1e_sb[:], t["W1e"][:, :])
        W2e_sb = cpool.tile([128, nslab, 64], BF16)
        nc.sync.dma_start(W2e_sb[:], t["W2e"][:, :, :])
        iota_sb = cpool.tile([128, 128], BF16)
        nc.sync.dma_start(iota_sb[:], t["iota"][:, :])
        ident_sb = cpool.tile([128, 128], F32)
        nc.sync.dma_start(ident_sb[:], t["ident"][:, :])
        ident_bf = cpool.tile([128, 128], BF16)
        nc.vector.tensor_copy(ident_bf[:], ident_sb[:])
        ald1_all = cpool.tile([128, NTB, H], F32)  # al_dst for every node tile
        ald1_sb = cpool.tile([128, nch, H], F32)  # this core's slab
        ald2_sb = cpool.tile([128, nch, 1], F32)  # L2 al_dst (ald_mode=tr)
        nc.vector.memset(ald1_all[:], 0.0)

        if cfg.get("phases", "full") == "none":
            return
        # ---------------- Phase 0: node table T1 ----------------
        es0 = nc.enter_named_scope("p0", False)
        with (
            tc.tile_pool(name="p0", bufs=3) as pool,
            tc.tile_pool(name="p0ps", bufs=2, space="PSUM") as pps,
        ):
            zt = pool.tile([128, ROW1], T1D, tag="zero")
            nc.vector.memset(zt[:], 0.0)
            for r0 in range(N, rows1, 128):
                nc.sync.dma_start(t["T1"][r0 : min(r0 + 128, rows1), :],
                                  zt[: min(128, rows1 - r0), :])

            ntile = -(-N // 128)
            for i in range(ntile):
                m = min(128, N - i * 128)
                xt = pool.tile([128, 128], BF16, tag="xt")
                nc.sync.dma_start(xt[:, :m], t["xT"][:, i * 128 : i * 128 + m])
                ps = pps.tile([128, 1024], F32, tag="ps")  # 2 banks
                nc.tensor.matmul(ps[:m, 0:HC], lhsT=xt[:, :m],
                                 rhs=W1e_sb[:, 0:HC], start=True, stop=True)
                nc.tensor.matmul(ps[:m, 512 : 512 + 2 * H], lhsT=xt[:, :m],
                                 rhs=W1e_sb[:, HC : HC + 2 * H],
                                 start=True, stop=True)
                # row [h1(512) | al_src packed as f32 bitcast]
                row = pool.tile([128, HC + nals], T1D, tag="row")
                nc.vector.tensor_copy(row[:m, 0 : HC // 2], ps[:m, 0 : HC // 2])
                nc.scalar.copy(row[:m, HC // 2 : HC], ps[:m, HC // 2 : HC])
                nc.vector.tensor_copy(row[:m, HC : HC + nals].bitcast(F32),
                                      ps[:m, 512 : 512 + H])
                # al_dst slab kept on-chip: tile i -> ald1_all[:, i, :]
                nc.scalar.copy(ald1_all[:m, i, :],
                               ps[:m, 512 + H : 512 + 2 * H])
                nc.sync.dma_start(
                    t["T1"][i * 128 : i * 128 + m, 0 : HC + nals], row[:m, :]
                )

        # this core's al_dst slab: columns [pid*nch, pid*nch + nch)
        pid = nc.partition_id()
        nc.sync.dma_start(
            ald1_sb[:], ald1_all[:, bass.ds(pid * nch, nch), :]
        )
        # dst table T_ald[npcp, 128]: own nodes' al_dst as f32 pairs
        with tc.tile_pool(name="tald", bufs=2) as apool:
            for c in range(nch):
                aldrow = apool.tile([128, 2 * H], BF16, tag="ar")
                nc.scalar.copy(aldrow[:].bitcast(F32), ald1_sb[:, c, :])
                nc.sync.dma_start(
                    t["T_ald"][c * 128 : (c + 1) * 128, 0 : 2 * H], aldrow[:]
                )
        nc.leave_named_scope("p0", es0[0], False)

        if cfg.get("phases", "full") == "p0":
            return
        # ---------------- L1 edge phase ----------------
        es1 = nc.enter_named_scope("l1", False)
        _edge_phase(
            tc, cfg, layer=1,
            gather_src=t["T1"], grow=ROW1, gcols=HC, gdt=T1D,
            als_cols=(HC, HC + nals),
            split=split1, rows=rows1,
            idx_d=t["g1idx"], gdidx_d=t["gdidx"], dl2_d=t["dl2"],
            dald_src=t["T_ald"], ald_cols=(0, 2 * H),
            k_lo=cfg["k1_lo"], k_hi=cfg["k1_hi"], toff=cfg["toff1"],
            iota_sb=iota_sb, ident_bf=ident_bf,
            W2e_sb=W2e_sb, tb2_own=t["tb2_own"], out2=None,
            H=H, nslab=nslab, NCLS_=NCLS,
            ald_sb=ald1_sb,
            ald2_cap=(ald2_sb if cfg.get("ald_mode", "gather") == "tr"
                      else None),
        )
        nc.leave_named_scope("l1", es1[0], False)

        if cfg.get("phases", "full") == "p0+l1":
            return
        # ---------------- allgather ----------------
        es_ag = nc.enter_named_scope("ag", False)
        if cfg.get("no_collective"):
            # timing-model builds only: stand-in DMA for the AllGather
            nc.sync.dma_start(t["tb2_full"][0:npcp, :], t["tb2_own"][:, :])
        else:
            nc.gpsimd.collective_compute(
                "AllGather",
                OP.bypass,
                replica_groups=[list(range(n_cores))],
                ins=[t["tb2_own"][:, :]],
                outs=[t["tb2_full"][:, :]],
            )
        nc.leave_named_scope("ag", es_ag[0], False)

        if cfg.get("phases", "full") == "p0+l1+ag":
            return
        # ---------------- L2 edge phase ----------------
        es2 = nc.enter_named_scope("l2", False)
        _edge_phase(
            tc, cfg, layer=2,
            gather_src=t["tb2_full"], grow=ROW2, gcols=NCLS, gdt=BF16,
            als_cols=(NCLS, NCLS + 2),
            split=split2, rows=rows2,
            idx_d=t["g1idx"], gdidx_d=t["gdidx"], dl2_d=t["dl2"],
            dald_src=t["tb2_own"], ald_cols=(NCLS + 2, NCLS + 4),
            k_lo=cfg["k2_lo"], k_hi=cfg["k2_hi"], toff=cfg["toff2"],
            iota_sb=iota_sb, ident_bf=ident_bf,
            W2e_sb=None, tb2_own=None, out2=t["out2"],
            H=1, nslab=nslab, ald_sb=ald2_sb,
        )
        nc.leave_named_scope("l2", es2[0], False)


def _edge_phase(tc, cfg, layer, gather_src, grow, gcols, gdt, als_cols,
                split, rows,
                idx_d, gdidx_d, dl2_d, dald_src, ald_cols, k_lo, k_hi, toff,
                iota_sb, ident_bf, W2e_sb, tb2_own, out2, H, nslab,
                NCLS_=None, ald_sb=None, ald2_cap=None):
    """One GAT message-passing layer over this core's dst chunks.

    The per-chunk output tail (softmax-normalize [+ W2 projection]) is
    software-pipelined one chunk behind the gather/aggregate main loop so
    the PE queue is never stalled by the DVE/Act tail chain.
    """
    nc = tc.nc
    nch = cfg["nch"]
    HC, NCLS = cfg["HC"], cfg["NCLS"]
    C = HC // H if layer == 1 else NCLS
    mode = cfg.get("edge_mode", "full")  # full | gather | gather_src
    ald_tr = cfg.get("ald_mode", "gather") == "tr"
    fp8 = gdt is FP8
    agg_w = gcols if layer == 1 else NCLS  # width of u-matmul rhs

    lo_ap = gather_src[0:split, :]
    hi_ap = gather_src[split:rows, :]
    GMAX = 8  # cap dma_gather at GMAX subtiles (runtime limit ~1400 idxs)
    nq = cfg.get("nq", 1)
    sp = cfg.get("sp", True)
    qctr = [0]

    def gq():
        q = qctr[0] % nq
        qctr[0] += 1
        return q

    with (
        tc.tile_pool(name=f"gt{layer}", bufs=2) as gpool,
        tc.tile_pool(name=f"ga{layer}", bufs=2) as gapool,
        tc.tile_pool(name=f"meta{layer}", bufs=2) as mpool,
        tc.tile_pool(name=f"sel{layer}", bufs=2) as spool,
        tc.tile_pool(name=f"msg{layer}", bufs=3) as msgpool,
        tc.tile_pool(name=f"small{layer}", bufs=3) as smpool,
        tc.tile_pool(name=f"out{layer}", bufs=2) as opool,
        tc.tile_pool(name=f"ps_tr{layer}", bufs=2, space="PSUM") as pp_tr,
        tc.tile_pool(name=f"ps_u{layer}", bufs=2, space="PSUM") as pp_u,
        tc.tile_pool(name=f"ps_z{layer}", bufs=1 if ald_tr else 2,
                     space="PSUM") as pp_z,
        tc.tile_pool(name=f"ps_o{layer}", bufs=1 if ald_tr else 2,
                     space="PSUM") as pp_o,
        tc.tile_pool(name=f"ps_ald{layer}", bufs=2, space="PSUM") as pp_ald,
    ):

        def chunk_main(c):
            klo, khi = k_lo[c], k_hi[c]
            CNT = klo + khi
            assert 1 <= CNT <= 64
            off = toff[c]

            idx = mpool.tile([128, 8 * CNT], I16, tag="idx")
            nc.sync.dma_start(idx[:], idx_d[:, 8 * off : 8 * (off + CNT)])
            if mode != "gather_src" and not ald_tr:
                didx = mpool.tile([128, 8 * CNT], I16, tag="didx")
                nc.sync.dma_start(didx[:],
                                  gdidx_d[:, 8 * off : 8 * (off + CNT)])
            if mode == "full":
                dl2 = mpool.tile([128, 2 * CNT], BF16, tag="dl2")
                nc.sync.dma_start(dl2[:], dl2_d[:, 2 * off : 2 * (off + CNT)])

            gt = gpool.tile([128, CNT, grow], gdt, tag="gt")
            for g0, g1, ap in ((0, klo, lo_ap), (klo, CNT, hi_ap)):
                for b0 in range(g0, g1, GMAX):
                    b1 = min(b0 + GMAX, g1)
                    nk = b1 - b0
                    nc.gpsimd.dma_gather(
                        gt[:, b0:b1, :], ap, idx[:, 8 * b0 : 8 * b1],
                        nk * 128, nk * 128, grow,
                        single_packet=sp, queue_num=gq(),
                    )
            if mode == "gather_src":
                return None
            if not ald_tr:
                ga = gapool.tile([128, CNT, 128], BF16, tag="ga")
                for b0 in range(0, CNT, GMAX):
                    b1 = min(b0 + GMAX, CNT)
                    nk = b1 - b0
                    nc.gpsimd.dma_gather(
                        ga[:, b0:b1, :], dald_src[:, :],
                        didx[:, 8 * b0 : 8 * b1],
                        nk * 128, nk * 128, 128,
                        single_packet=sp, queue_num=gq(),
                    )
            if mode != "full":
                return None

            # one-hot selection: sel[e, tt, d] = (dl[e,tt] == d), 2x DVE mode
            sel = spool.tile([128, CNT, 128], BF16, tag="sel")
            nc.vector.tensor_tensor(
                sel[:].rearrange("p t (q r) -> p t q r", r=2),
                dl2[:].rearrange("p (t r) -> p t r", r=2)[:, :, None, :]
                .to_broadcast([128, CNT, 64, 2]),
                iota_sb[:].rearrange("p (q r) -> p q r", r=2)[:, None, :, :]
                .to_broadcast([128, CNT, 64, 2]),
                op=OP.is_equal,
            )

            # p = exp(leakyrelu(al_src + al_dst))
            if ald_tr:
                # per-edge al_dst via transposed selection matrices
                ps_ald = pp_ald.tile([128, CNT, H], F32, tag="ald")
                for tt in range(CNT):
                    ps_tr = pp_tr.tile([128, 128], BF16, tag="tr")
                    nc.tensor.transpose(ps_tr[:], sel[:, tt, :], ident_bf[:])
                    seldm = smpool.tile([128, 128], F32, tag="seldm")
                    nc.scalar.copy(seldm[:], ps_tr[:])
                    nc.tensor.matmul(
                        ps_ald[:, tt, :], lhsT=seldm[:], rhs=ald_sb[:, c, :],
                        start=True, stop=True,
                    )
                alde = ps_ald[:]
            else:
                alde = ga[:, :, ald_cols[0] : ald_cols[1]].bitcast(F32)
            s_t = smpool.tile([128, CNT, H], F32, tag="s")
            nc.vector.tensor_tensor(
                s_t[:],
                gt[:, :, als_cols[0] : als_cols[1]].bitcast(F32),
                alde,
                op=OP.add,
            )
            l_t = smpool.tile([128, CNT, H], F32, tag="l")
            nc.vector.scalar_tensor_tensor(
                l_t[:], s_t[:], 0.2, s_t[:], op0=OP.mult, op1=OP.max
            )
            pb = smpool.tile([128, CNT, H], BF16, tag="pb")
            nc.scalar.activation(pb[:], l_t[:], AF.Exp)
            if not fp8:
                p2 = smpool.tile([128, CNT, H, 2], BF16, tag="p2")
                nc.vector.tensor_copy(
                    p2[:], pb[:, :, :, None].to_broadcast([128, CNT, H, 2])
                )

            ps_u = pp_u.tile([128, 512], F32, tag="u")
            ps_z = pp_z.tile([128, H], F32, tag="z")
            for tt in range(CNT):
                msg = msgpool.tile([128, agg_w], BF16, tag="msg")
                if layer == 1 and fp8:
                    nc.vector.tensor_tensor(
                        msg[:].rearrange("p (h c) -> p h c", h=H),
                        gt[:, tt, 0:gcols].rearrange("p (h c) -> p h c", h=H),
                        pb[:, tt, :, None].to_broadcast([128, H, C]),
                        op=OP.mult,
                    )
                elif layer == 1:
                    nc.vector.tensor_tensor(
                        msg[:].rearrange("p (h q r) -> p h q r", h=H, r=2),
                        gt[:, tt, 0:gcols].rearrange("p (h q r) -> p h q r",
                                                     h=H, r=2),
                        p2[:, tt, :, None, :].to_broadcast([128, H, C // 2, 2]),
                        op=OP.mult,
                    )
                else:
                    nc.vector.tensor_tensor(
                        msg[:, 0:NCLS].rearrange("p (q r) -> p q r", r=2),
                        gt[:, tt, 0:NCLS].rearrange("p (q r) -> p q r", r=2),
                        p2[:, tt, 0, None, :].to_broadcast([128, NCLS // 2, 2]),
                        op=OP.mult,
                    )
                nc.tensor.matmul(
                    ps_u[:, 0:agg_w], lhsT=sel[:, tt, :], rhs=msg[:, 0:agg_w],
                    start=(tt == 0), stop=(tt == CNT - 1),
                )
                nc.tensor.matmul(
                    ps_z[:], lhsT=sel[:, tt, :], rhs=pb[:, tt, :],
                    start=(tt == 0), stop=(tt == CNT - 1),
                )
            return (c, ps_u, ps_z)

        def chunk_tail(st):
            c, ps_u, ps_z = st
            if layer == 1:
                zb = smpool.tile([128, H], F32, tag="zb")
                nc.vector.tensor_scalar_max(zb[:], ps_z[:], 1e-30)
                rz = smpool.tile([128, H], F32, tag="rz")
                nc.vector.reciprocal(rz[:], zb[:])
                h2 = opool.tile([128, HC], F32, tag="h2")
                nc.vector.tensor_tensor(
                    h2[:].rearrange("p (h c) -> p h c", h=H),
                    ps_u[:, 0:HC].rearrange("p (h c) -> p h c", h=H),
                    rz[:, :, None].to_broadcast([128, H, C]),
                    op=OP.mult,
                )
                h2r = opool.tile([128, HC], BF16, tag="h2r")
                nc.scalar.activation(h2r[:], h2[:], AF.Relu)
                # fused W2_ext projection -> tb2 row
                ps_o = pp_o.tile([128, 64], F32, tag="o")
                for j in range(nslab):
                    ps_tr = pp_tr.tile([128, 128], BF16, tag="tr")
                    nc.tensor.transpose(
                        ps_tr[:], h2r[:, j * 128 : (j + 1) * 128], ident_bf[:]
                    )
                    h2t = smpool.tile([128, 128], BF16, tag="h2t")
                    nc.scalar.copy(h2t[:], ps_tr[:])
                    nc.tensor.matmul(
                        ps_o[:], lhsT=h2t[:], rhs=W2e_sb[:, j, :],
                        start=(j == 0), stop=(j == nslab - 1),
                    )
                trow = opool.tile([128, 128], BF16, tag="trow")
                nc.vector.tensor_copy(trow[:, 0:NCLS_], ps_o[:, 0:NCLS_])
                nc.scalar.copy(
                    trow[:, NCLS_ : NCLS_ + 4].bitcast(F32),
                    ps_o[:, NCLS_ : NCLS_ + 2],
                )
                if ald2_cap is not None:
                    nc.scalar.copy(ald2_cap[:, c, :],
                                   ps_o[:, NCLS_ + 1 : NCLS_ + 2])
                nc.sync.dma_start(tb2_own[c * 128 : (c + 1) * 128, :], trow[:])
            else:
                zb = smpool.tile([128, 1], F32, tag="zb")
                nc.vector.tensor_scalar_max(zb[:], ps_z[:], 1e-30)
                rz = smpool.tile([128, 1], F32, tag="rz")
                nc.vector.reciprocal(rz[:], zb[:])
                o2 = opool.tile([128, NCLS], F32, tag="o2")
                nc.vector.tensor_tensor(
                    o2[:], ps_u[:, 0:NCLS],
                    rz[:].to_broadcast([128, NCLS]), op=OP.mult,
                )
                nc.sync.dma_start(out2[c * 128 : (c + 1) * 128, :], o2[:])

        prev = None
        for c in range(nch):
            st = chunk_main(c)
            if prev is not None:
                chunk_tail(prev)
            prev = st
        if prev is not None:
            chunk_tail(prev)


# ----------------------------------------------------------------------------
# PJRT execution (with optional on-device iteration chaining for timing)
# ----------------------------------------------------------------------------

def _pjrt_exec(nc, in_maps, n_cores, iters=1, reps=3):
    """Like bass2jax.run_bass_via_pjrt but chains `iters` sequential
    executions of the NEFF inside one jit (iteration i+1 consumes iteration
    i's outputs as its donated output buffers), so (t[K]-t[1])/(K-1) measures
    pure on-device kernel time without host/transfer overhead."""
    import jax
    import numpy as _np
    from jax.sharding import Mesh, PartitionSpec
    from jax.experimental.shard_map import shard_map
    from concourse import bass2jax as b2j
    from concourse import mybir as _mb

    b2j.install_neuronx_cc_hook()
    partition_name = (nc.partition_id_tensor.name
                      if nc.partition_id_tensor else None)
    in_names, out_names, out_avals, zero_outs = [], [], [], []
    for alloc in nc.m.functions[0].allocations:
        if not isinstance(alloc, _mb.MemoryLocationSet):
            continue
        name = alloc.memorylocations[0].name
        if alloc.kind == "ExternalInput":
            if name != partition_name:
                in_names.append(name)
        elif alloc.kind == "ExternalOutput":
            shape = tuple(alloc.tensor_shape)
            dtype = _mb.dt.np(alloc.dtype)
            out_names.append(name)
            out_avals.append(jax.core.ShapedArray(shape, dtype))
            zero_outs.append(_np.zeros(shape, dtype))
    n_params = len(in_names)
    all_in_names = in_names + out_names
    if partition_name is not None:
        all_in_names = all_in_names + [partition_name]

    def _body(*args):
        ins = list(args[:n_params])
        zo = list(args[n_params:])
        for _ in range(iters):
            operands = ins + zo
            if partition_name is not None:
                operands.append(b2j.partition_id_tensor())
            outs = _bass_exec_bind(b2j, operands, out_avals, all_in_names,
                                   out_names, nc)
            zo = list(outs)
        return tuple(zo)

    devices = jax.devices()[:n_cores]
    mesh = Mesh(_np.asarray(devices), ("core",))
    in_specs = (PartitionSpec("core"),) * (n_params + len(out_names))
    out_specs = (PartitionSpec("core"),) * len(out_names)
    sharded = jax.jit(shard_map(_body, mesh=mesh, in_specs=in_specs,
                                out_specs=out_specs, check_rep=False),
                      keep_unused=True)
    concat_in = [
        _np.concatenate([_np.asarray(in_maps[c][nm]) for c in range(n_cores)],
                        axis=0)
        for nm in in_names
    ]
    concat_zeros = [_np.zeros((n_cores * z.shape[0], *z.shape[1:]), z.dtype)
                    for z in zero_outs]
    import time as _time
    from jax.sharding import NamedSharding
    sh = NamedSharding(mesh, PartitionSpec("core"))
    dev_in = [jax.device_put(a, sh) for a in concat_in]
    dev_zeros = [jax.device_put(a, sh) for a in concat_zeros]
    jax.block_until_ready(dev_in + dev_zeros)
    out_arrs = sharded(*dev_in, *dev_zeros)  # compile + run
    jax.block_until_ready(out_arrs)
    times = []
    for _ in range(reps):
        t0 = _time.perf_counter()
        out_arrs = sharded(*dev_in, *dev_zeros)
        jax.block_until_ready(out_arrs)
        times.append(_time.perf_counter() - t0)
    dt = min(times)
    results = [
        {nm: _np.asarray(out_arrs[i]).reshape(n_cores, *out_avals[i].shape)[c]
         for i, nm in enumerate(out_names)}
        for c in range(n_cores)
    ]
    return results, dt


def _bass_exec_bind(b2j, operands, out_avals, in_names, out_names, nc):
    return b2j._bass_exec_p.bind(
        *operands,
        out_avals=tuple(out_avals),
        in_names=tuple(in_names),
        out_names=tuple(out_names),
        lowering_input_output_aliases=(),
        sim_require_finite=True,
        sim_require_nnan=True,
        nc=nc,
    )


# ----------------------------------------------------------------------------
# Entry point
# ----------------------------------------------------------------------------

_CACHE = {}


def _run(inputs, trace=False):
    x = np.asarray(inputs["x"], np.float32)
    edge_index = np.asarray(inputs["edge_index"], np.int32)
    W1 = np.asarray(inputs["W1"], np.float32)
    a1s = np.asarray(inputs["att1_src"], np.float32)
    a1d = np.asarray(inputs["att1_dst"], np.float32)
    W2 = np.asarray(inputs["W2"], np.float32)
    a2s = np.asarray(inputs["att2_src"], np.float32)
    a2d = np.asarray(inputs["att2_dst"], np.float32)
    b1 = np.asarray(inputs["b1"], np.float32)
    b2 = np.asarray(inputs["b2"], np.float32)
    assert not b1.any() and not b2.any(), "nonzero bias unsupported"

    key = hashlib.sha1(
        b"v2" + edge_index.tobytes() + np.int64(x.shape).tobytes()
    ).hexdigest()
    cfg, shared, per_core = _host_prep(x, edge_index, W1, a1s, a1d, W2, a2s, a2d)
    if key not in _CACHE:
        _CACHE[key] = _build_program(cfg)
    nc = _CACHE[key]

    in_maps = []
    for k in range(cfg["n_cores"]):
        m = dict(shared)
        m.update(per_core[k])
        in_maps.append(m)
    res = run_bass_kernel_spmd(nc, in_maps, list(range(cfg["n_cores"])),
                               trace=trace)
    out = gather_out([res.results[k]["out2"] for k in range(cfg["n_cores"])],
                     cfg)
    return out.astype(np.float32), res


def gather_out(outs, cfg):
    N, npcp = cfg["N"], cfg["npcp"]
    return np.concatenate(
        [outs[k][: min(npcp, N - k * npcp)] for k in range(cfg["n_cores"])],
        axis=0,
    )


def kernel(**inputs):
    out, _ = _run(inputs, trace=False)
    return out


# revision 24
# speedup vs baseline: 2.3501x; 1.1609x over previous
"""GAT (2-layer, PyG-default) Trainium2 Bass kernel, 8-core SPMD.

Strategy:
  - Destinations (and their incoming edges) are partitioned across the 8
    cores: core k owns dst nodes [k*npc, (k+1)*npc).
  - Phase 0 (replicated on every core): h1 = x @ [W1 | W1@Asrc | W1@Adst]
    written to a DRAM node table T1[rows, 640] = [h1(512) | al_src f32
    pairs(16) | pad], batched 4 node-tiles per DMA.  Replicating this
    matmul avoids a 100MB allgather.  A small per-core dst table
    T_ald[npcp, 128] holds each OWN node's al_dst (f32 pairs, cols 0:16).
  - L1 edge phase: edges are grouped by dst into chunks of 128 consecutive
    dst nodes; per chunk bulk dma_gathers pull T1[src] for all its edges
    (1280B/edge) and T_ald[dst] (256B/edge, chunk-local page-friendly
    indices, no int16 split).  Gathers round-robin over 4 SWDGE queues
    (num_swdge_queues=4), which roughly doubles gather throughput and
    makes the dst stream nearly free.  One-hot selection matrices
    sel[e,d] (built on DVE in the 2x 16-bit mode from pair-duplicated
    dst-local ids) route per-edge messages into PSUM via PE matmuls:
        u[d,:] = sum_e sel[e,d] * p[e,h] * h1[src_e]      (unnormalized)
        z[d,h] = sum_e sel[e,d] * p[e,h]
    with p = exp(leakyrelu(al_src[src] + al_dst[dst])), the msg multiply
    also in the DVE 2x mode via pair-duplicated p.  Softmax is done
    unnormalized (shift-invariance not needed in fp32 at these logit
    magnitudes) and normalized once per dst: out = u / z.  The per-chunk
    output tail (normalize, relu, fused W2_ext projection into a tb2 row
    [h2b(40) | al2 f32 pairs(4) | pad]) is software-pipelined one chunk
    behind the main loop so the PE queue never stalls on it.
  - One AllGather shares the [npcp,128] per-core tb2 tables (~58us on 4
    queues); the L2 edge phase repeats the same machinery with 256B/edge
    gathers, a single head, and tb2_own as its dst table.
  - int16 gather indices can't span 50k rows, so each chunk's edges are
    split into a low-half / high-half group (by table row), each gathered
    with a different base AP.  The dst gathers are chunk-local and need
    no split.

Self-contained: only needs numpy + the concourse tree at /opt/trn_rl_repo.
"""

import hashlib
import math
import os
import sys

import numpy as np

for _p in ("/opt/trn_rl_repo",):
    if _p not in sys.path:
        sys.path.insert(0, _p)

import concourse.bacc as bacc
import concourse.bass as bass
import concourse.tile as tile
from concourse import mybir
from concourse.bass_utils import run_bass_kernel_spmd

F32 = mybir.dt.float32
BF16 = mybir.dt.bfloat16
FP8 = mybir.dt.float8e4
I16 = mybir.dt.int16
AF = mybir.ActivationFunctionType
OP = mybir.AluOpType

N_CORES = 8
_PAD_NEG = False  # pad gather slots use idx -1 (skip fetch) when True


# ----------------------------------------------------------------------------
# Host-side edge planning
# ----------------------------------------------------------------------------

def _edge_plan(src_rows, dst, n_cores, npc, nch, split, nrows):
    """Group edges by (dst-core, dst-chunk-of-128, src-half) and lay out
    gather indices / dst-local arrays.

    src_rows: int64 [E] table row per edge.  dst: int64 [E] global dst.
    split: table rows >= split are gathered from a base-offset AP so the
    local index fits int16.

    Returns (k_lo[nch], k_hi[nch], toff[nch], TOT,
             idx16 [n_cores,128,8*TOT] int16,    # src-gather indices
             gd16  [n_cores,128,8*TOT] int16,    # dst-gather indices (dloc)
             dl2   [n_cores,128,2*TOT] f32).     # pair-duplicated dst-local
    Token t*128+p of a chunk lives at partition p, free col toff+t.
    Padding tokens gather row 0 and have dl = -1 (matches no dst).
    """
    core = dst // npc
    dloc = dst - core * npc
    chunk = dloc >> 7
    d128 = dloc & 127
    half = (src_rows >= split).astype(np.int64)
    lidx = src_rows - half * split
    assert lidx.min() >= 0 and lidx.max() < 32768
    assert dloc.max() < 32768

    ngr = n_cores * nch * 2
    key = (core * nch + chunk) * 2 + half
    cnt = np.bincount(key, minlength=ngr).reshape(n_cores, nch, 2)
    kg = -(-cnt // 128)
    k_lo = kg[:, :, 0].max(axis=0)
    k_hi = kg[:, :, 1].max(axis=0)
    CNT = k_lo + k_hi
    toff = np.zeros(nch, np.int64)
    toff[1:] = np.cumsum(CNT)[:-1]
    TOT = int(CNT.sum())

    order = np.argsort(key, kind="stable")
    sk = key[order]
    gstart = np.zeros(ngr + 1, np.int64)
    np.cumsum(np.bincount(sk, minlength=ngr), out=gstart[1:])
    rank = np.arange(len(sk)) - gstart[sk]

    c_ = chunk[order]
    h_ = half[order]
    co_ = core[order]
    bs = toff[c_] + np.where(h_ == 1, k_lo[c_], 0)  # block start (128-token units)
    pos = bs * 128 + rank  # token position within the core's full layout

    dl = np.full((n_cores, 128, TOT), -1.0, np.float32)
    dl[co_, pos % 128, pos // 128] = d128[order].astype(np.float32)
    dl2 = np.repeat(dl, 2, axis=2)  # [n_cores, 128, 2*TOT] pair-duplicated

    j = rank  # token index local to this gather block
    # Pad slots get idx -1: trailing negatives are skipped by the gather
    # (no fetch, SBUF slot keeps old finite data).  Chunks 0/1 read
    # never-written pool bufs, so their pads fetch row 0 instead.
    idx16 = np.full((n_cores, 16, 8 * TOT), -1, np.int16)
    idx16[co_, j % 16, 8 * bs + j // 16] = lidx[order].astype(np.int16)
    head = 8 * (toff[2] if nch > 2 and _PAD_NEG else TOT)
    r = idx16[:, :, :head]
    r[r == -1] = 0
    idx16 = np.tile(idx16, (1, 8, 1))
    gd16 = np.zeros((n_cores, 16, 8 * TOT), np.int16)
    gd16[co_, j % 16, 8 * bs + j // 16] = dloc[order].astype(np.int16)
    gd16 = np.tile(gd16, (1, 8, 1))

    return (
        [int(v) for v in k_lo],
        [int(v) for v in k_hi],
        [int(v) for v in toff],
        TOT,
        idx16,
        gd16,
        dl2,
    )


def _host_prep(x, edge_index, W1, att1_src, att1_dst, W2, att2_src, att2_dst):
    N, F = x.shape
    H, C = att1_src.shape
    HC = H * C
    NCLS = W2.shape[1]
    n_cores = N_CORES
    # 128-aligned dst partition: core k owns [k*npcp, (k+1)*npcp) ∩ [0, N)
    nch = -(-N // (n_cores * 128))
    npcp = nch * 128
    npc = npcp
    assert (n_cores - 1) * npcp < N <= n_cores * npcp

    src = np.concatenate([edge_index[0], np.arange(N, dtype=edge_index.dtype)])
    dst = np.concatenate([edge_index[1], np.arange(N, dtype=edge_index.dtype)])
    src = src.astype(np.int64)
    dst = dst.astype(np.int64)

    split1 = (N // 2 + 127) & ~127
    rows1 = n_cores * npcp  # >= N; pad rows zeroed on device
    plan1 = _edge_plan(src, dst, n_cores, npc, nch, split1, rows1)

    rows2 = n_cores * npcp
    split2 = (n_cores // 2) * npcp
    assert rows2 == rows1 and split2 == split1
    # identity row map: L2 gather plan == L1 plan

    # Folded attention-logit weight columns: al_src = x @ (W1 @ blockdiag(a))
    Wa_s = np.einsum("fhc,hc->fh", W1.reshape(F, H, C), att1_src).astype(np.float32)
    Wa_d = np.einsum("fhc,hc->fh", W1.reshape(F, H, C), att1_dst).astype(np.float32)
    W1e = np.ascontiguousarray(
        np.concatenate([W1, Wa_s, Wa_d], axis=1), dtype=np.float32
    )  # [F, HC+2H]

    w2s = (W2 @ att2_src[0]).astype(np.float32)  # [HC]
    w2d = (W2 @ att2_dst[0]).astype(np.float32)
    W2e_flat = np.zeros((HC, 64), np.float32)
    W2e_flat[:, :NCLS] = W2
    W2e_flat[:, NCLS] = w2s
    W2e_flat[:, NCLS + 1] = w2d
    nslab = HC // 128
    W2e = np.ascontiguousarray(
        W2e_flat.reshape(nslab, 128, 64).transpose(1, 0, 2)
    )  # [128, nslab, 64]

    import ml_dtypes
    bf = ml_dtypes.bfloat16
    xT = np.ascontiguousarray(x.T).astype(bf)  # [F, N]
    W1e = W1e.astype(bf)
    iota = np.tile(np.arange(128, dtype=np.float32), (128, 1)).astype(bf)
    ident = np.eye(128, dtype=np.float32)

    cfg = dict(
        nq=4,
        N=N, F=F, H=H, C=C, HC=HC, NCLS=NCLS, n_cores=n_cores, npc=npc,
        nch=nch, npcp=npcp, split1=split1, rows1=rows1, split2=split2,
        rows2=rows2, nslab=nslab,
        k1_lo=plan1[0], k1_hi=plan1[1], toff1=plan1[2], TOT1=plan1[3],
        k2_lo=plan1[0], k2_hi=plan1[1], toff2=plan1[2], TOT2=plan1[3],
    )
    shared = dict(xT=xT, W1e=W1e, W2e=W2e.astype(bf), iota=iota, ident=ident)
    per_core = [
        dict(g1idx=plan1[4][k], gdidx=plan1[5][k], dl2=plan1[6][k].astype(bf))
        for k in range(n_cores)
    ]
    return cfg, shared, per_core


# ----------------------------------------------------------------------------
# Device program
# ----------------------------------------------------------------------------

def _build_program(cfg):
    N, F, H, HC, NCLS = cfg["N"], cfg["F"], cfg["H"], cfg["HC"], cfg["NCLS"]
    n_cores, nch, npcp = cfg["n_cores"], cfg["nch"], cfg["npcp"]
    rows1, split1 = cfg["rows1"], cfg["split1"]
    rows2, split2 = cfg["rows2"], cfg["split2"]
    nslab = cfg["nslab"]
    fp8 = cfg.get("t1_fp8", False)
    ROW1 = 768 if fp8 else 640  # T1 row: [h1(512) | al_src f32 bitcast | pad]
    ROW2 = 128  # bf16 cols: [h2b(40) | al2 f32 pairs(4) | pad]
    assert F == 128 and HC % 128 == 0

    nc = bacc.Bacc("TRN2", target_bir_lowering=False, debug=False,
                   num_devices=n_cores,
                   num_swdge_queues=cfg.get("nq", 1))

    xT = nc.dram_tensor("xT", [F, N], BF16, kind="ExternalInput").ap()
    W1e = nc.dram_tensor("W1e", [F, HC + 2 * H], BF16, kind="ExternalInput").ap()
    W2e = nc.dram_tensor("W2e", [128, nslab, 64], BF16, kind="ExternalInput").ap()
    iota_d = nc.dram_tensor("iota", [128, 128], BF16, kind="ExternalInput").ap()
    ident_d = nc.dram_tensor("ident", [128, 128], F32, kind="ExternalInput").ap()
    g1idx = nc.dram_tensor("g1idx", [128, 8 * cfg["TOT1"]], I16,
                           kind="ExternalInput").ap()
    gdidx = nc.dram_tensor("gdidx", [128, 8 * cfg["TOT1"]], I16,
                           kind="ExternalInput").ap()
    dl2_d = nc.dram_tensor("dl2", [128, 2 * cfg["TOT1"]], BF16,
                           kind="ExternalInput").ap()

    T1 = nc.dram_tensor("T1", [rows1, ROW1], FP8 if fp8 else BF16).ap()
    T_ald = nc.dram_tensor("T_ald", [npcp, ROW2], BF16).ap()
    tb2_own = nc.dram_tensor("tb2_own", [npcp, ROW2], BF16).ap()
    tb2_full = nc.dram_tensor("tb2_full", [rows2, ROW2], BF16,
                              addr_space="Shared").ap()
    out2 = nc.dram_tensor("out2", [npcp, NCLS], F32, kind="ExternalOutput").ap()

    tensors = dict(
        xT=xT, W1e=W1e, W2e=W2e, iota=iota_d, ident=ident_d,
        g1idx=g1idx, gdidx=gdidx, dl2=dl2_d,
        T1=T1, T_ald=T_ald, tb2_own=tb2_own, tb2_full=tb2_full, out2=out2,
    )
    repeat = cfg.get("repeat", 1)
    with tile.TileContext(nc) as tc:
        for _ in range(repeat):
            _emit(tc, cfg, tensors)
    nc.compile()
    return nc


def _emit(tc, cfg, t):
    nc = tc.nc
    N, F, H, HC, NCLS = cfg["N"], cfg["F"], cfg["H"], cfg["HC"], cfg["NCLS"]
    n_cores, nch, npc, npcp = cfg["n_cores"], cfg["nch"], cfg["npc"], cfg["npcp"]
    rows1, split1 = cfg["rows1"], cfg["split1"]
    rows2, split2 = cfg["rows2"], cfg["split2"]
    nslab = cfg["nslab"]
    fp8 = cfg.get("t1_fp8", False)
    ROW1, ROW2 = (768 if fp8 else 640), 128
    T1D = FP8 if fp8 else BF16
    nals = 4 * H if fp8 else 2 * H  # al_src f32 span in T1-dtype units
    NW1 = HC + 2 * H  # phase-0 matmul width

    NTB = n_cores * nch  # total 128-row tiles across the padded table
    with tc.tile_pool(name="consts", bufs=1) as cpool:
        W1e_sb = cpool.tile([128, NW1], BF16)
        nc.sync.dma_start(W1e_sb[:], t["W1e"][:, :])
        W2e_sb = cpool.tile([128, nslab, 64], BF16)
        nc.sync.dma_start(W2e_sb[:], t["W2e"][:, :, :])
        iota_sb = cpool.tile([128, 128], BF16)
        nc.sync.dma_start(iota_sb[:], t["iota"][:, :])
        ident_sb = cpool.tile([128, 128], F32)
        nc.sync.dma_start(ident_sb[:], t["ident"][:, :])
        ident_bf = cpool.tile([128, 128], BF16)
        nc.vector.tensor_copy(ident_bf[:], ident_sb[:])
        ald1_all = cpool.tile([128, NTB, H], F32)  # al_dst for every node tile
        ald1_sb = cpool.tile([128, nch, H], F32)  # this core's slab
        ald2_sb = cpool.tile([128, nch, 1], F32)  # L2 al_dst (ald_mode=tr)
        nc.vector.memset(ald1_all[:], 0.0)

        if cfg.get("phases", "full") == "none":
            return
        # ---------------- Phase 0: node table T1 ----------------
        es0 = nc.enter_named_scope("p0", False)
        with (
            tc.tile_pool(name="p0", bufs=3) as pool,
            tc.tile_pool(name="p0ps", bufs=2, space="PSUM") as pps,
        ):
            zt = pool.tile([128, ROW1], T1D, tag="zero")
            nc.vector.memset(zt[:], 0.0)
            for r0 in range(N, rows1, 128):
                nc.sync.dma_start(t["T1"][r0 : min(r0 + 128, rows1), :],
                                  zt[: min(128, rows1 - r0), :])

            # batches of NB node-tiles per DMA (cuts SP dma_start issue cost)
            NB = 4
            ntile = -(-N // 128)
            ngrp = -(-ntile // NB)
            for g in range(ngrp):
                i0 = g * NB
                nb = min(NB, ntile - i0)
                mm = min(NB * 128, N - i0 * 128)  # valid nodes in group
                xt = pool.tile([128, NB * 128], BF16, tag="xt")
                nc.sync.dma_start(xt[:, :mm],
                                  t["xT"][:, i0 * 128 : i0 * 128 + mm])
                row = pool.tile([128, NB, HC + nals], T1D, tag="row")
                p0_mode = cfg.get("p0_mode", "full")
                for j in range(nb if p0_mode != "dma" else 0):
                    i = i0 + j
                    m = min(128, N - i * 128)
                    ps = pps.tile([128, 1024], F32, tag="ps")  # 2 banks
                    nc.tensor.matmul(ps[:m, 0:HC],
                                     lhsT=xt[:, j * 128 : j * 128 + m],
                                     rhs=W1e_sb[:, 0:HC], start=True, stop=True)
                    nc.tensor.matmul(ps[:m, 512 : 512 + 2 * H],
                                     lhsT=xt[:, j * 128 : j * 128 + m],
                                     rhs=W1e_sb[:, HC : HC + 2 * H],
                                     start=True, stop=True)
                    if p0_mode == "mm":
                        continue
                    nc.vector.tensor_copy(row[:m, j, 0 : HC // 2],
                                          ps[:m, 0 : HC // 2])
                    nc.scalar.copy(row[:m, j, HC // 2 : HC],
                                   ps[:m, HC // 2 : HC])
                    nc.vector.tensor_copy(
                        row[:m, j, HC : HC + nals].bitcast(F32),
                        ps[:m, 512 : 512 + H])
                    # al_dst slab kept on-chip: tile i -> ald1_all[:, i, :]
                    nc.scalar.copy(ald1_all[:m, i, :],
                                   ps[:m, 512 + H : 512 + 2 * H])
                mr = min(NB * 128, rows1 - i0 * 128)
                nc.sync.dma_start(
                    t["T1"][i0 * 128 : i0 * 128 + mr, 0 : HC + nals]
                    .rearrange("(j p) c -> p j c", p=128)[:, :nb, :],
                    row[:, :nb, :],
                )

        # this core's al_dst slab: columns [pid*nch, pid*nch + nch)
        pid = nc.partition_id()
        nc.sync.dma_start(
            ald1_sb[:], ald1_all[:, bass.ds(pid * nch, nch), :]
        )
        # dst table T_ald[npcp, 128]: own nodes' al_dst as f32 pairs
        with tc.tile_pool(name="tald", bufs=2) as apool:
            for c in range(nch):
                aldrow = apool.tile([128, 2 * H], BF16, tag="ar")
                nc.scalar.copy(aldrow[:].bitcast(F32), ald1_sb[:, c, :])
                nc.sync.dma_start(
                    t["T_ald"][c * 128 : (c + 1) * 128, 0 : 2 * H], aldrow[:]
                )
        nc.leave_named_scope("p0", es0[0], False)

        if cfg.get("phases", "full") == "p0":
            return
        # ---------------- L1 edge phase ----------------
        es1 = nc.enter_named_scope("l1", False)
        _edge_phase(
            tc, cfg, layer=1,
            gather_src=t["T1"], grow=ROW1, gcols=HC, gdt=T1D,
            als_cols=(HC, HC + nals),
            split=split1, rows=rows1,
            idx_d=t["g1idx"], gdidx_d=t["gdidx"], dl2_d=t["dl2"],
            dald_src=t["T_ald"], ald_cols=(0, 2 * H),
            k_lo=cfg["k1_lo"], k_hi=cfg["k1_hi"], toff=cfg["toff1"],
            iota_sb=iota_sb, ident_bf=ident_bf,
            W2e_sb=W2e_sb, tb2_own=t["tb2_own"], out2=None,
            H=H, nslab=nslab, NCLS_=NCLS,
            ald_sb=ald1_sb,
            ald2_cap=(ald2_sb if cfg.get("ald_mode", "gather") == "tr"
                      else None),
        )
        nc.leave_named_scope("l1", es1[0], False)

        if cfg.get("phases", "full") == "p0+l1":
            return
        # ---------------- allgather ----------------
        es_ag = nc.enter_named_scope("ag", False)
        if cfg.get("no_collective"):
            # timing-model builds only: stand-in DMA for the AllGather
            nc.sync.dma_start(t["tb2_full"][0:npcp, :], t["tb2_own"][:, :])
        else:
            nc.gpsimd.collective_compute(
                "AllGather",
                OP.bypass,
                replica_groups=[list(range(n_cores))],
                ins=[t["tb2_own"][:, :]],
                outs=[t["tb2_full"][:, :]],
            )
        nc.leave_named_scope("ag", es_ag[0], False)

        if cfg.get("phases", "full") == "p0+l1+ag":
            return
        # ---------------- L2 edge phase ----------------
        es2 = nc.enter_named_scope("l2", False)
        _edge_phase(
            tc, cfg, layer=2,
            gather_src=t["tb2_full"], grow=ROW2, gcols=NCLS, gdt=BF16,
            als_cols=(NCLS, NCLS + 2),
            split=split2, rows=rows2,
            idx_d=t["g1idx"], gdidx_d=t["gdidx"], dl2_d=t["dl2"],
            dald_src=t["tb2_own"], ald_cols=(NCLS + 2, NCLS + 4),
            k_lo=cfg["k2_lo"], k_hi=cfg["k2_hi"], toff=cfg["toff2"],
            iota_sb=iota_sb, ident_bf=ident_bf,
            W2e_sb=None, tb2_own=None, out2=t["out2"],
            H=1, nslab=nslab, ald_sb=ald2_sb,
        )
        nc.leave_named_scope("l2", es2[0], False)


def _edge_phase(tc, cfg, layer, gather_src, grow, gcols, gdt, als_cols,
                split, rows,
                idx_d, gdidx_d, dl2_d, dald_src, ald_cols, k_lo, k_hi, toff,
                iota_sb, ident_bf, W2e_sb, tb2_own, out2, H, nslab,
                NCLS_=None, ald_sb=None, ald2_cap=None):
    """One GAT message-passing layer over this core's dst chunks.

    The per-chunk output tail (softmax-normalize [+ W2 projection]) is
    software-pipelined one chunk behind the gather/aggregate main loop so
    the PE queue is never stalled by the DVE/Act tail chain.
    """
    nc = tc.nc
    nch = cfg["nch"]
    HC, NCLS = cfg["HC"], cfg["NCLS"]
    C = HC // H if layer == 1 else NCLS
    mode = cfg.get("edge_mode", "full")  # full | gather | gather_src
    ald_tr = cfg.get("ald_mode", "gather") == "tr"
    fp8 = gdt is FP8
    agg_w = gcols if layer == 1 else NCLS  # width of u-matmul rhs

    lo_ap = gather_src[0:split, :]
    hi_ap = gather_src[split:rows, :]
    GMAX = cfg.get("gmax", 8)  # dma_gather subtile cap (runtime limit ~1400 idxs)
    nq = cfg.get("nq", 1)
    sp = cfg.get("sp", True)
    qctr = [0]

    def gq():
        q = qctr[0] % nq
        qctr[0] += 1
        return q

    gb = cfg.get("gtbufs", 2)
    with (
        tc.tile_pool(name=f"gt{layer}", bufs=gb) as gpool,
        tc.tile_pool(name=f"ga{layer}", bufs=gb) as gapool,
        tc.tile_pool(name=f"meta{layer}", bufs=gb) as mpool,
        tc.tile_pool(name=f"sel{layer}", bufs=2) as spool,
        tc.tile_pool(name=f"msg{layer}", bufs=3) as msgpool,
        tc.tile_pool(name=f"small{layer}", bufs=3) as smpool,
        tc.tile_pool(name=f"out{layer}", bufs=2) as opool,
        tc.tile_pool(name=f"ps_tr{layer}", bufs=2, space="PSUM") as pp_tr,
        tc.tile_pool(name=f"ps_u{layer}", bufs=2, space="PSUM") as pp_u,
        tc.tile_pool(name=f"ps_z{layer}", bufs=1 if ald_tr else 2,
                     space="PSUM") as pp_z,
        tc.tile_pool(name=f"ps_o{layer}", bufs=1 if ald_tr else 2,
                     space="PSUM") as pp_o,
        tc.tile_pool(name=f"ps_ald{layer}", bufs=2, space="PSUM") as pp_ald,
    ):

        def chunk_main(c):
            klo, khi = k_lo[c], k_hi[c]
            CNT = klo + khi
            assert 1 <= CNT <= 64
            off = toff[c]

            idx = mpool.tile([128, 8 * CNT], I16, tag="idx")
            nc.sync.dma_start(idx[:], idx_d[:, 8 * off : 8 * (off + CNT)])
            if mode != "gather_src" and not ald_tr:
                didx = mpool.tile([128, 8 * CNT], I16, tag="didx")
                nc.sync.dma_start(didx[:],
                                  gdidx_d[:, 8 * off : 8 * (off + CNT)])
            if mode == "full":
                dl2 = mpool.tile([128, 2 * CNT], BF16, tag="dl2")
                nc.sync.dma_start(dl2[:], dl2_d[:, 2 * off : 2 * (off + CNT)])

            gt = gpool.tile([128, CNT, grow], gdt, tag="gt")
            for g0, g1, ap in ((0, klo, lo_ap), (klo, CNT, hi_ap)):
                for b0 in range(g0, g1, GMAX):
                    b1 = min(b0 + GMAX, g1)
                    nk = b1 - b0
                    nc.gpsimd.dma_gather(
                        gt[:, b0:b1, :], ap, idx[:, 8 * b0 : 8 * b1],
                        nk * 128, nk * 128, grow,
                        single_packet=sp, queue_num=gq(),
                    )
            if mode == "gather_src":
                return None
            if not ald_tr:
                ga = gapool.tile([128, CNT, 128], BF16, tag="ga")
                for b0 in range(0, CNT, GMAX):
                    b1 = min(b0 + GMAX, CNT)
                    nk = b1 - b0
                    nc.gpsimd.dma_gather(
                        ga[:, b0:b1, :], dald_src[:, :],
                        didx[:, 8 * b0 : 8 * b1],
                        nk * 128, nk * 128, 128,
                        single_packet=sp, queue_num=gq(),
                    )
            if mode != "full":
                return None

            # one-hot selection: sel[e, tt, d] = (dl[e,tt] == d), 2x DVE mode
            sel = spool.tile([128, CNT, 128], BF16, tag="sel")
            nc.vector.tensor_tensor(
                sel[:].rearrange("p t (q r) -> p t q r", r=2),
                dl2[:].rearrange("p (t r) -> p t r", r=2)[:, :, None, :]
                .to_broadcast([128, CNT, 64, 2]),
                iota_sb[:].rearrange("p (q r) -> p q r", r=2)[:, None, :, :]
                .to_broadcast([128, CNT, 64, 2]),
                op=OP.is_equal,
            )

            # p = exp(leakyrelu(al_src + al_dst))
            if ald_tr:
                # per-edge al_dst via transposed selection matrices
                ps_ald = pp_ald.tile([128, CNT, H], F32, tag="ald")
                for tt in range(CNT):
                    ps_tr = pp_tr.tile([128, 128], BF16, tag="tr")
                    nc.tensor.transpose(ps_tr[:], sel[:, tt, :], ident_bf[:])
                    seldm = smpool.tile([128, 128], F32, tag="seldm")
                    nc.scalar.copy(seldm[:], ps_tr[:])
                    nc.tensor.matmul(
                        ps_ald[:, tt, :], lhsT=seldm[:], rhs=ald_sb[:, c, :],
                        start=True, stop=True,
                    )
                alde = ps_ald[:]
            else:
                alde = ga[:, :, ald_cols[0] : ald_cols[1]].bitcast(F32)
            s_t = smpool.tile([128, CNT, H], F32, tag="s")
            nc.vector.tensor_tensor(
                s_t[:],
                gt[:, :, als_cols[0] : als_cols[1]].bitcast(F32),
                alde,
                op=OP.add,
            )
            l_t = smpool.tile([128, CNT, H], F32, tag="l")
            nc.vector.scalar_tensor_tensor(
                l_t[:], s_t[:], 0.2, s_t[:], op0=OP.mult, op1=OP.max
            )
            pb = smpool.tile([128, CNT, H], BF16, tag="pb")
            nc.scalar.activation(pb[:], l_t[:], AF.Exp)
            if not fp8:
                p2 = smpool.tile([128, CNT, H, 2], BF16, tag="p2")
                nc.vector.tensor_copy(
                    p2[:], pb[:, :, :, None].to_broadcast([128, CNT, H, 2])
                )

            ps_u = pp_u.tile([128, 512], F32, tag="u")
            ps_z = pp_z.tile([128, H], F32, tag="z")
            for tt in range(CNT):
                msg = msgpool.tile([128, agg_w], BF16, tag="msg")
                if layer == 1 and fp8:
                    nc.vector.tensor_tensor(
                        msg[:].rearrange("p (h c) -> p h c", h=H),
                        gt[:, tt, 0:gcols].rearrange("p (h c) -> p h c", h=H),
                        pb[:, tt, :, None].to_broadcast([128, H, C]),
                        op=OP.mult,
                    )
                elif layer == 1:
                    nc.vector.tensor_tensor(
                        msg[:].rearrange("p (h q r) -> p h q r", h=H, r=2),
                        gt[:, tt, 0:gcols].rearrange("p (h q r) -> p h q r",
                                                     h=H, r=2),
                        p2[:, tt, :, None, :].to_broadcast([128, H, C // 2, 2]),
                        op=OP.mult,
                    )
                else:
                    nc.vector.tensor_tensor(
                        msg[:, 0:NCLS].rearrange("p (q r) -> p q r", r=2),
                        gt[:, tt, 0:NCLS].rearrange("p (q r) -> p q r", r=2),
                        p2[:, tt, 0, None, :].to_broadcast([128, NCLS // 2, 2]),
                        op=OP.mult,
                    )
                nc.tensor.matmul(
                    ps_u[:, 0:agg_w], lhsT=sel[:, tt, :], rhs=msg[:, 0:agg_w],
                    start=(tt == 0), stop=(tt == CNT - 1),
                )
                nc.tensor.matmul(
                    ps_z[:], lhsT=sel[:, tt, :], rhs=pb[:, tt, :],
                    start=(tt == 0), stop=(tt == CNT - 1),
                )
            return (c, ps_u, ps_z)

        def chunk_tail(st):
            c, ps_u, ps_z = st
            if layer == 1:
                zb = smpool.tile([128, H], F32, tag="zb")
                nc.vector.tensor_scalar_max(zb[:], ps_z[:], 1e-30)
                rz = smpool.tile([128, H], F32, tag="rz")
                nc.vector.reciprocal(rz[:], zb[:])
                h2 = opool.tile([128, HC], F32, tag="h2")
                nc.vector.tensor_tensor(
                    h2[:].rearrange("p (h c) -> p h c", h=H),
                    ps_u[:, 0:HC].rearrange("p (h c) -> p h c", h=H),
                    rz[:, :, None].to_broadcast([128, H, C]),
                    op=OP.mult,
                )
                h2r = opool.tile([128, HC], BF16, tag="h2r")
                nc.scalar.activation(h2r[:], h2[:], AF.Relu)
                # fused W2_ext projection -> tb2 row
                ps_o = pp_o.tile([128, 64], F32, tag="o")
                for j in range(nslab):
                    ps_tr = pp_tr.tile([128, 128], BF16, tag="tr")
                    nc.tensor.transpose(
                        ps_tr[:], h2r[:, j * 128 : (j + 1) * 128], ident_bf[:]
                    )
                    h2t = smpool.tile([128, 128], BF16, tag="h2t")
                    nc.scalar.copy(h2t[:], ps_tr[:])
                    nc.tensor.matmul(
                        ps_o[:], lhsT=h2t[:], rhs=W2e_sb[:, j, :],
                        start=(j == 0), stop=(j == nslab - 1),
                    )
                trow = opool.tile([128, 128], BF16, tag="trow")
                nc.vector.tensor_copy(trow[:, 0:NCLS_], ps_o[:, 0:NCLS_])
                nc.scalar.copy(
                    trow[:, NCLS_ : NCLS_ + 4].bitcast(F32),
                    ps_o[:, NCLS_ : NCLS_ + 2],
                )
                if ald2_cap is not None:
                    nc.scalar.copy(ald2_cap[:, c, :],
                                   ps_o[:, NCLS_ + 1 : NCLS_ + 2])
                nc.sync.dma_start(tb2_own[c * 128 : (c + 1) * 128, :], trow[:])
            else:
                zb = smpool.tile([128, 1], F32, tag="zb")
                nc.vector.tensor_scalar_max(zb[:], ps_z[:], 1e-30)
                rz = smpool.tile([128, 1], F32, tag="rz")
                nc.vector.reciprocal(rz[:], zb[:])
                o2 = opool.tile([128, NCLS], F32, tag="o2")
                nc.vector.tensor_tensor(
                    o2[:], ps_u[:, 0:NCLS],
                    rz[:].to_broadcast([128, NCLS]), op=OP.mult,
                )
                nc.sync.dma_start(out2[c * 128 : (c + 1) * 128, :], o2[:])

        prev = None
        for c in range(nch):
            st = chunk_main(c)
            if prev is not None:
                chunk_tail(prev)
            prev = st
        if prev is not None:
            chunk_tail(prev)


# ----------------------------------------------------------------------------
# PJRT execution (with optional on-device iteration chaining for timing)
# ----------------------------------------------------------------------------

def _pjrt_exec(nc, in_maps, n_cores, iters=1, reps=3):
    """Like bass2jax.run_bass_via_pjrt but chains `iters` sequential
    executions of the NEFF inside one jit (iteration i+1 consumes iteration
    i's outputs as its donated output buffers), so (t[K]-t[1])/(K-1) measures
    pure on-device kernel time without host/transfer overhead."""
    import jax
    import numpy as _np
    from jax.sharding import Mesh, PartitionSpec
    from jax.experimental.shard_map import shard_map
    from concourse import bass2jax as b2j
    from concourse import mybir as _mb

    b2j.install_neuronx_cc_hook()
    partition_name = (nc.partition_id_tensor.name
                      if nc.partition_id_tensor else None)
    in_names, out_names, out_avals, zero_outs = [], [], [], []
    for alloc in nc.m.functions[0].allocations:
        if not isinstance(alloc, _mb.MemoryLocationSet):
            continue
        name = alloc.memorylocations[0].name
        if alloc.kind == "ExternalInput":
            if name != partition_name:
                in_names.append(name)
        elif alloc.kind == "ExternalOutput":
            shape = tuple(alloc.tensor_shape)
            dtype = _mb.dt.np(alloc.dtype)
            out_names.append(name)
            out_avals.append(jax.core.ShapedArray(shape, dtype))
            zero_outs.append(_np.zeros(shape, dtype))
    n_params = len(in_names)
    all_in_names = in_names + out_names
    if partition_name is not None:
        all_in_names = all_in_names + [partition_name]

    def _body(*args):
        ins = list(args[:n_params])
        zo = list(args[n_params:])
        for _ in range(iters):
            operands = ins + zo
            if partition_name is not None:
                operands.append(b2j.partition_id_tensor())
            outs = _bass_exec_bind(b2j, operands, out_avals, all_in_names,
                                   out_names, nc)
            zo = list(outs)
        return tuple(zo)

    devices = jax.devices()[:n_cores]
    mesh = Mesh(_np.asarray(devices), ("core",))
    in_specs = (PartitionSpec("core"),) * (n_params + len(out_names))
    out_specs = (PartitionSpec("core"),) * len(out_names)
    sharded = jax.jit(shard_map(_body, mesh=mesh, in_specs=in_specs,
                                out_specs=out_specs, check_rep=False),
                      keep_unused=True)
    concat_in = [
        _np.concatenate([_np.asarray(in_maps[c][nm]) for c in range(n_cores)],
                        axis=0)
        for nm in in_names
    ]
    concat_zeros = [_np.zeros((n_cores * z.shape[0], *z.shape[1:]), z.dtype)
                    for z in zero_outs]
    import time as _time
    from jax.sharding import NamedSharding
    sh = NamedSharding(mesh, PartitionSpec("core"))
    dev_in = [jax.device_put(a, sh) for a in concat_in]
    dev_zeros = [jax.device_put(a, sh) for a in concat_zeros]
    jax.block_until_ready(dev_in + dev_zeros)
    out_arrs = sharded(*dev_in, *dev_zeros)  # compile + run
    jax.block_until_ready(out_arrs)
    times = []
    for _ in range(reps):
        t0 = _time.perf_counter()
        out_arrs = sharded(*dev_in, *dev_zeros)
        jax.block_until_ready(out_arrs)
        times.append(_time.perf_counter() - t0)
    dt = min(times)
    results = [
        {nm: _np.asarray(out_arrs[i]).reshape(n_cores, *out_avals[i].shape)[c]
         for i, nm in enumerate(out_names)}
        for c in range(n_cores)
    ]
    return results, dt


def _bass_exec_bind(b2j, operands, out_avals, in_names, out_names, nc):
    return b2j._bass_exec_p.bind(
        *operands,
        out_avals=tuple(out_avals),
        in_names=tuple(in_names),
        out_names=tuple(out_names),
        lowering_input_output_aliases=(),
        sim_require_finite=True,
        sim_require_nnan=True,
        nc=nc,
    )


# ----------------------------------------------------------------------------
# Entry point
# ----------------------------------------------------------------------------

_CACHE = {}


def _run(inputs, trace=False):
    x = np.asarray(inputs["x"], np.float32)
    edge_index = np.asarray(inputs["edge_index"], np.int32)
    W1 = np.asarray(inputs["W1"], np.float32)
    a1s = np.asarray(inputs["att1_src"], np.float32)
    a1d = np.asarray(inputs["att1_dst"], np.float32)
    W2 = np.asarray(inputs["W2"], np.float32)
    a2s = np.asarray(inputs["att2_src"], np.float32)
    a2d = np.asarray(inputs["att2_dst"], np.float32)
    b1 = np.asarray(inputs["b1"], np.float32)
    b2 = np.asarray(inputs["b2"], np.float32)
    assert not b1.any() and not b2.any(), "nonzero bias unsupported"

    key = hashlib.sha1(
        b"v2" + edge_index.tobytes() + np.int64(x.shape).tobytes()
    ).hexdigest()
    cfg, shared, per_core = _host_prep(x, edge_index, W1, a1s, a1d, W2, a2s, a2d)
    if key not in _CACHE:
        _CACHE[key] = _build_program(cfg)
    nc = _CACHE[key]

    in_maps = []
    for k in range(cfg["n_cores"]):
        m = dict(shared)
        m.update(per_core[k])
        in_maps.append(m)
    res = run_bass_kernel_spmd(nc, in_maps, list(range(cfg["n_cores"])),
                               trace=trace)
    out = gather_out([res.results[k]["out2"] for k in range(cfg["n_cores"])],
                     cfg)
    return out.astype(np.float32), res


def gather_out(outs, cfg):
    N, npcp = cfg["N"], cfg["npcp"]
    return np.concatenate(
        [outs[k][: min(npcp, N - k * npcp)] for k in range(cfg["n_cores"])],
        axis=0,
    )


def kernel(**inputs):
    out, _ = _run(inputs, trace=False)
    return out
